# revision 65
# baseline (speedup 1.0000x reference)
"""ConvCNP1d Trainium2 kernel, v4.

Banded RBF via host-side sorting (ls = ln2 over a 128-unit range means
entries vanish beyond |d| ~ 2.7; output is un-sorted on the host).

Key structure (see v2/v3 history in git-less comments):
- RBF exponents a*(x-t)^2 are built entirely by one PE matmul per tile
  from hi/lo-split fp16 rank-1 rows (squared terms + cross term), then a
  single fused Exp emits the fp16 K tile.  No per-chunk DVE work.
- Encoder runs on 16 value-blocks of 128 grid points (narrow bands =>
  fewer padded (xc, t) pairs, and [128, <=512B] PSUM tiles so the eps
  pool can quadruple-buffer).  Decoder runs on 8 xt quantile-blocks of
  256 targets against fixed grid chunks.
- Conv decoder is batch-fused (block-diagonal weights process both
  per-core batches in one matmul) with taps folded into the partition
  dim via shifted stack copies at 32-aligned partition bases; tap 4 is
  a second matmul reading the base block at a column offset.  conv1's
  t channel is affine in the grid index: two static hi/lo t rows + a
  bias + an exact 4-column edge correction added into PSUM.
- h0/h1 epilogue folds h into [8, 256] tiles (DMA gather) so the
  reciprocal/ratio run wide, then DMA scatters into the conv1 stack.
- DMA descriptor generation on the sync engine (~0.6us per dma_start)
  is a hidden serializer: inputs are packed into 6 loads split across
  the two HWDGE queues (sync + scalar), outputs accumulate into one
  [2, 2048] tile per batch and leave in one DMA each.
"""

import numpy as np

T_GRID = 2048
B = 16
N = 2048
NCORES = 8
BLOC = B // NCORES
NBLK_E = 16
WBLK_E = T_GRID // NBLK_E   # 128
NBLK_D = 8
TGTU = T_GRID // NBLK_D     # 256
ETH = 7.5                   # exponent cutoff; entries below e^-ETH dropped
RD = 12                     # decoder kgen rows (2 + 5*2 per half)
TP = T_GRID + 8             # padded stack width (data at col j+4-o)

_PROG_CACHE = {}


def build_program(cfg):
    import concourse.bacc as bacc
    import concourse.tile as tile
    from concourse import mybir

    f32 = mybir.dt.float32
    f16 = mybir.dt.float16
    AF = mybir.ActivationFunctionType
    OP = mybir.AluOpType

    NCH_E = cfg["NCH_E"]
    NCH_D = cfg["NCH_D"]
    J0S = cfg["J0S"]
    os_rho = cfg["os_rho"]
    b4_0 = cfg["b4_0"]
    b4_1 = cfg["b4_1"]
    SE = sum(NCH_E)
    MAXNE = max(NCH_E)
    MAXND = max(NCH_D)
    RE = 2 + 5 * MAXNE
    BW = MAXNE * WBLK_E                      # BDE col width
    KGWC = BW + 2 * NBLK_D * 128             # KGW cols
    XBC = NBLK_E * 128 + NBLK_D * 2 * TGTU   # XB cols
    assert MAXNE * WBLK_E <= 512 and MAXND <= 4

    nc = bacc.Bacc(None, target_bir_lowering=False)

    KGWh = nc.declare_dram_parameter("KGW", [17, KGWC], f16, isOutput=False)
    XBh = nc.declare_dram_parameter("XB", [BLOC, 17, XBC], f16, isOutput=False)
    PHIh = nc.declare_dram_parameter("PHI", [BLOC, 128, 2 * SE], f16, isOutput=False)
    WALLh = nc.declare_dram_parameter("WALL", [128, 332], f16, isOutput=False)
    BALLh = nc.declare_dram_parameter("BALL", [64, 8], f32, isOutput=False)
    TROWh = nc.declare_dram_parameter("TROW", [2, T_GRID], f16, isOutput=False)
    OUTh = nc.declare_dram_parameter("out", [BLOC, 2, T_GRID], f32, isOutput=True)

    with tile.TileContext(nc) as tc:
        with (
            tc.tile_pool(name="singles", bufs=1) as singles,
            tc.tile_pool(name="perb", bufs=2) as perb,
            tc.tile_pool(name="kpool", bufs=4) as kpool,
            tc.tile_pool(name="k2keep", bufs=1) as k2keep,
            tc.tile_pool(name="small", bufs=1) as small,
            tc.tile_pool(name="psE", bufs=2, space="PSUM") as psE,
            tc.tile_pool(name="psD", bufs=2, space="PSUM") as psD,
            tc.tile_pool(name="psC", bufs=2, space="PSUM") as psC,
        ):
            # ---- loads: split into queue-parallel pieces; a small first
            # piece covers the critical path (enc block 0 + PHI + TDB) ----
            KGW = singles.tile([17, KGWC], f16)
            st = [dict() for _ in range(BLOC)]
            for b in range(BLOC):
                s = st[b]
                s["XB"] = perb.tile([17, XBC], f16, tag="XB", name="XB_sb")
                s["PHI"] = perb.tile([128, 2 * SE], f16, tag="PHI", name="PHI_sb")
                s["h"] = perb.tile([2, T_GRID], f32, tag="h_sb", name="h_sb")
                for hf, w in (("A", 144), ("B", 112)):
                    s[f"hg0{hf}"] = perb.tile([8, w], f32, tag=f"hg0{hf}",
                                              name=f"hg0{hf}")
                    s[f"hg1{hf}"] = perb.tile([8, w], f32, tag=f"hg1{hf}",
                                              name=f"hg1{hf}")
                    s[f"rec{hf}"] = perb.tile([8, w], f32, tag=f"rec{hf}",
                                              name=f"rec{hf}")
                    s[f"h0f{hf}"] = perb.tile([8, w], f16, tag=f"h0f{hf}",
                                              name=f"h0f{hf}")
                    s[f"ratf{hf}"] = perb.tile([8, w], f16, tag=f"ratf{hf}",
                                               name=f"ratf{hf}")
                s["fT"] = perb.tile([128, 2, 16], f16, tag="fT", name="fT")
                s["osl"] = perb.tile([2, T_GRID], f32, tag="osl", name="osl")
            NE = NBLK_E * 128
            nc.sync.dma_start(out=KGW[0:17, 0:256], in_=KGWh[0:17, 0:256])
            nc.sync.dma_start(out=st[0]["XB"][0:17, 0:256],
                              in_=XBh[0, 0:17, 0:256])
            if BW > 256:
                nc.sync.dma_start(out=KGW[0:17, 256:BW],
                                  in_=KGWh[0:17, 256:BW])
            nc.sync.dma_start(out=st[0]["XB"][0:17, 256:512],
                              in_=XBh[0, 0:17, 256:512])
            nc.sync.dma_start(out=st[0]["PHI"][:, 0:SE], in_=PHIh[0, :, 0:SE])
            nc.sync.dma_start(out=st[0]["PHI"][:, SE:2 * SE],
                              in_=PHIh[0, :, SE:2 * SE])
            for c0 in range(512, NE, 512):
                nc.sync.dma_start(out=st[0]["XB"][0:17, c0:c0 + 512],
                                  in_=XBh[0, 0:17, c0:c0 + 512])
            nc.sync.dma_start(out=KGW[0:12, BW:BW + 1024],
                              in_=KGWh[0:12, BW:BW + 1024])
            nc.sync.dma_start(out=KGW[0:12, BW + 1024:KGWC],
                              in_=KGWh[0:12, BW + 1024:KGWC])
            for c0 in range(NE, XBC, 1024):
                nc.sync.dma_start(out=st[0]["XB"][0:12, c0:c0 + 1024],
                                  in_=XBh[0, 0:12, c0:c0 + 1024])
            # batch-1 inputs via gpsimd (idle early; keeps the scalar queue
            # free for Exps), small singles appended to the sync queue.
            # Each piece is gated on enc(0) block 0's h so the transfers
            # don't contend with the critical-path head DMAs.
            def gated_b1_load(rows, c0):
                nc.vector.tensor_copy(st[1]["XB"][0:1, c0:c0 + 1],
                                      st[0]["h"][0:1, 0:1])
                nc.gpsimd.dma_start(out=st[1]["XB"][0:rows, c0:c0 + 1024],
                                    in_=XBh[1, 0:rows, c0:c0 + 1024])

            for c0 in range(0, NE, 1024):
                gated_b1_load(17, c0)
            for c0 in range(NE, XBC, 1024):
                gated_b1_load(12, c0)
            nc.sync.dma_start(out=st[1]["PHI"][:, 0:SE],
                              in_=PHIh[1, :, 0:SE])
            nc.sync.dma_start(out=st[1]["PHI"][:, SE:2 * SE],
                              in_=PHIh[1, :, SE:2 * SE])
            WALL = singles.tile([128, 332], f16)
            nc.sync.dma_start(out=WALL, in_=WALLh[:, :])
            BALL = singles.tile([64, 8], f32)
            nc.sync.dma_start(out=BALL, in_=BALLh[:, :])

            def bde(rows, c0, c1):
                return KGW[0:rows, c0:c1]

            def tdb(half, rows, k):
                o = BW + 1024 * half
                return KGW[0:rows, o + 128 * k:o + 128 * (k + 1)]

            def xcb(b, rows, k):
                return st[b]["XB"][0:rows, 128 * k:128 * (k + 1)]

            def xtq(b, rows, k, tot):
                o = NBLK_E * 128 + 2 * TGTU * k
                return st[b]["XB"][0:rows, o:o + tot]

            W1n = WALL[0:100, 0:32]
            W1e = WALL[0:4, 32:64]
            W2n = WALL[0:128, 64:128]
            W2e = WALL[0:32, 128:192]
            W3n = WALL[0:128, 192:224]
            W3b = WALL[0:128, 224:256]
            W3e4 = WALL[0:64, 256:288]
            W4n = WALL[0:128, 320:324]
            W4e = WALL[0:32, 324:328]
            ID4 = WALL[0:4, 328:332]
            B2a = BALL[0:64, 0:1]
            B3a = BALL[0:32, 1:2]
            C1a = BALL[0:32, 2:3]
            CRa = BALL[0:32, 3:7]

            # conv stacks (shared by both batches; taps in partition blocks;
            # block 0 rows 0-3 = data so shift copies read from base 0,
            # rows 4-5 of block 0 = the static affine t rows)
            C1S = singles.tile([100, TP], f16)
            nc.vector.memset(C1S, 0.0)
            nc.sync.dma_start(out=C1S[4:6, 2:2 + T_GRID], in_=TROWh[:, :])
            F2 = singles.tile([128, TP], f16)   # 4 taps x (16ch x 2b)
            F3 = singles.tile([128, TP], f16)   # taps 0,1 x (32ch x 2b)
            F3B = singles.tile([128, TP], f16)  # taps 2,3 x (32ch x 2b)
            F4 = singles.tile([128, TP], f16)   # 4 taps x (16ch x 2b)
            for F, blk in ((F2, 32), (F3, 64), (F3B, 64), (F4, 32)):
                for o in range(128 // blk):
                    nc.vector.memset(F[blk * o:blk * o + blk, 0:4], 0.0)
                    nc.vector.memset(F[blk * o:blk * o + blk, T_GRID:TP], 0.0)
            FRAW = singles.tile([4, T_GRID], f16)  # b0mu,b0sg,b1mu,b1sg

            def enc_block(b, k):
                s = st[b]
                nch = NCH_E[k]
                base = sum(NCH_E[:k])
                rows = 2 + 5 * nch
                tot = nch * WBLK_E
                eps = psE.tile([128, BW], f32, tag="E", name="E_ps")
                nc.tensor.matmul(eps[:, 0:tot], xcb(b, rows, k),
                                 bde(rows, 0, tot), start=True, stop=True)
                kt = kpool.tile([128, BW], f16, tag="K", name="K1t")
                nc.scalar.activation(out=kt[:, 0:tot], in_=eps[:, 0:tot],
                                     func=AF.Exp)
                hps = psC.tile([2, TGTU], f32, tag="c", name="h_ps")
                for c in range(nch):
                    nc.tensor.matmul(
                        hps[:, 0:WBLK_E],
                        s["PHI"][:, 2 * (base + c):2 * (base + c) + 2],
                        kt[:, WBLK_E * c:WBLK_E * (c + 1)],
                        start=(c == 0), stop=(c == nch - 1),
                    )
                nc.vector.tensor_copy(
                    s["h"][:, WBLK_E * k:WBLK_E * (k + 1)], hps[:, 0:WBLK_E])

            def dec_block(b, k):
                # both window-halves matmul into one 2-bank PSUM tile so a
                # single fused Exp emits the whole block's K tile
                s = st[b]
                nch = NCH_D[k]
                tot = nch * TGTU
                s[f"k2t_{k}"] = k2keep.tile(
                    [128, MAXND * TGTU], f16, tag=f"k2_{b}_{k}",
                    name=f"k2_{b}_{k}")
                eps = psD.tile([128, MAXND * TGTU], f32, tag="D", name="D_ps")
                # gate: tiny WAW dep on enc(0)'s finished h keeps the sim
                # scheduler from hoisting dec K-tiles ahead of the encoder
                # (where they stall the in-order PE queue on XTQ DMAs)
                nc.vector.tensor_copy(eps[0:2, 0:1],
                                      st[0]["h"][0:2, T_GRID - 1:T_GRID])
                nc.tensor.matmul(eps[:, 0:512], tdb(0, 12, k),
                                 xtq(b, 12, k, 512), start=True, stop=True)
                if tot > 512:
                    rows1 = 2 + 5 * (nch - 2)
                    nc.tensor.matmul(eps[:, 512:tot], tdb(1, rows1, k),
                                     xtq(b, rows1, k, tot - 512),
                                     start=True, stop=True)
                nc.scalar.activation(out=s[f"k2t_{k}"][:, 0:tot],
                                     in_=eps[:, 0:tot], func=AF.Exp)

            def epilogue(b, hf):
                # half-pipelined (A = grid cols 0:1152, B = 1152:2048) so the
                # conv1 stack's first chunks are ready before the encoder
                # finishes; gpsimd-issued DMAs keep descriptor generation off
                # the busy HWDGE queues
                s = st[b]
                o, w = (0, 1152) if hf == "A" else (1152, 896)
                nc.gpsimd.dma_start(out=s[f"hg0{hf}"],
                                    in_=s["h"][0:1, o:o + w])
                nc.gpsimd.dma_start(out=s[f"hg1{hf}"],
                                    in_=s["h"][1:2, o:o + w])
                nc.vector.reciprocal_approx_fast(s[f"rec{hf}"], s[f"hg0{hf}"])
                nc.vector.tensor_mul(s[f"ratf{hf}"], s[f"hg1{hf}"],
                                     s[f"rec{hf}"])
                nc.vector.tensor_copy(s[f"h0f{hf}"], s[f"hg0{hf}"])
                nc.gpsimd.dma_start(
                    out=C1S[2 * b:2 * b + 1, 4 + o:4 + o + w],
                    in_=s[f"h0f{hf}"])
                nc.gpsimd.dma_start(
                    out=C1S[2 * b + 1:2 * b + 2, 4 + o:4 + o + w],
                    in_=s[f"ratf{hf}"])

            def conv_chunk(l, n):
                c0 = 512 * n
                if l == 0:
                    ps = psC.tile([32, 512], f32, tag="c", name="c_ps")
                    nc.tensor.matmul(ps, W1n, C1S[:, 2 + c0:2 + c0 + 512],
                                     start=True, stop=False)
                    nc.tensor.matmul(ps, W1e, C1S[0:4, 6 + c0:6 + c0 + 512],
                                     start=False, stop=True)
                    if n == 0:
                        nc.vector.tensor_add(ps[:, 0:2], ps[:, 0:2], CRa[:, 0:2])
                    if n == 3:
                        nc.vector.tensor_add(ps[:, 510:512], ps[:, 510:512],
                                             CRa[:, 2:4])
                    nc.scalar.activation(out=F2[0:32, 4 + c0:4 + c0 + 512],
                                         in_=ps, func=AF.Relu, bias=C1a)
                elif l == 1:
                    ps = psC.tile([64, 512], f32, tag="c", name="c_ps")
                    nc.tensor.matmul(ps, W2n, F2[:, 2 + c0:2 + c0 + 512],
                                     start=True, stop=False)
                    nc.tensor.matmul(ps, W2e, F2[0:32, 6 + c0:6 + c0 + 512],
                                     start=False, stop=True)
                    nc.scalar.activation(out=F3[0:64, 4 + c0:4 + c0 + 512],
                                         in_=ps, func=AF.Relu, bias=B2a)
                elif l == 2:
                    ps = psC.tile([32, 512], f32, tag="c", name="c_ps")
                    nc.tensor.matmul(ps, W3n, F3[:, 2 + c0:2 + c0 + 512],
                                     start=True, stop=False)
                    nc.tensor.matmul(ps, W3b, F3B[:, 2 + c0:2 + c0 + 512],
                                     start=False, stop=False)
                    nc.tensor.matmul(ps, W3e4,
                                     F3[0:64, 6 + c0:6 + c0 + 512],
                                     start=False, stop=True)
                    nc.scalar.activation(out=F4[0:32, 4 + c0:4 + c0 + 512],
                                         in_=ps, func=AF.Relu, bias=B3a)
                else:
                    ps = psC.tile([4, 512], f32, tag="c", name="c_ps")
                    nc.tensor.matmul(ps, W4n, F4[:, 2 + c0:2 + c0 + 512],
                                     start=True, stop=False)
                    nc.tensor.matmul(ps, W4e, F4[0:32, 6 + c0:6 + c0 + 512],
                                     start=False, stop=True)
                    nc.vector.tensor_copy(FRAW[:, c0:c0 + 512], ps)

            def stack_shift(F, blk, rows, n):
                # per-chunk tap-block shifts (+4 col overlap so block-o reads
                # never need the next chunk's shift) let the next layer start
                # before this layer's later chunks finish
                c0 = 512 * n
                w = min(516, TP - 4 - c0)
                for o in range(1, 128 // blk):
                    nc.gpsimd.tensor_copy(
                        F[blk * o:blk * o + rows, 4 + c0 - o:4 + c0 - o + w],
                        F[0:rows, 4 + c0:4 + c0 + w])

            def shift_f3(n):
                c0 = 512 * n
                w = min(516, TP - 4 - c0)
                src = F3[0:64, 4 + c0:4 + c0 + w]
                nc.gpsimd.tensor_copy(
                    F3[64:128, 3 + c0:3 + c0 + w], src)
                nc.gpsimd.tensor_copy(
                    F3B[0:64, 2 + c0:2 + c0 + w], src)
                nc.gpsimd.tensor_copy(
                    F3B[64:128, 1 + c0:1 + c0 + w], src)

            # softplus via exp + ln1p(u) minimax poly keeps Scalar inside
            # the Exp/Relu act table (no mid-program ACT_TABLE_LOAD)
            LN1P = (-0.055459313742069534, 0.21866548366220714,
                    -0.46644243862756585, 0.9962619482337954,
                    6.944574454161809e-05)
            MUB = singles.tile([128, 16], f32)
            nc.vector.memset(MUB, float(os_rho * b4_0))

            def fchain(b, half):
                # per-half (grid chunks 0:8 / 8:16) so the first dec_mms can
                # overlap conv4's later chunks
                s = st[b]
                j0 = 8 * half
                ftp = psC.tile([128, 32], f16, tag="c", name="ftp")
                for j in range(8):
                    nc.tensor.transpose(
                        ftp[:, 4 * j:4 * j + 4],
                        FRAW[:, 128 * (j0 + j):128 * (j0 + j + 1)],
                        ID4)
                mu = ftp[:, 2 * b::4]
                sg = ftp[:, 2 * b + 1::4]
                fsl = slice(j0, j0 + 8)
                x = small.tile([128, 8], f32, tag="t1", name="t1")
                e = small.tile([128, 8], f32, tag="t2", name="t2")
                p = small.tile([128, 8], f32, tag="t3", name="t3")
                r = small.tile([128, 8], f32, tag="t4", name="t4")
                nc.vector.scalar_tensor_tensor(
                    s["fT"][:, 0, fsl], mu, float(os_rho), MUB[:, 0:8],
                    OP.mult, OP.add)
                nc.vector.tensor_scalar_add(x, sg, float(b4_1))
                nc.vector.scalar_tensor_tensor(e, x, -1.0, x, OP.mult, OP.min)
                nc.scalar.activation(out=e, in_=e, func=AF.Exp)
                # os_rho * ln1p(e) via nested Horner, one STT per step
                cs = [float(os_rho * c) for c in LN1P]
                nc.vector.tensor_scalar_mul(p, e, cs[0])
                for ck in cs[1:-1]:
                    nc.vector.scalar_tensor_tensor(p, p, ck, e,
                                                   OP.add, OP.mult)
                nc.vector.tensor_scalar_add(p, p, cs[-1])
                nc.vector.tensor_scalar(r, x, float(os_rho), 0.0,
                                        OP.mult, OP.max)
                nc.vector.tensor_add(s["fT"][:, 1, fsl], p, r)

            def dec_mm(b, k):
                s = st[b]
                kt = s[f"k2t_{k}"]
                nch = NCH_D[k]
                msps = psD.tile([2, TGTU], f32, tag="D", name="ms_ps")
                for c in range(nch):
                    nc.tensor.matmul(
                        msps,
                        s["fT"][:, :, J0S[k] + c],
                        kt[:, TGTU * c:TGTU * (c + 1)],
                        start=(c == 0), stop=(c == nch - 1),
                    )
                nc.vector.tensor_copy(
                    s["osl"][:, TGTU * k:TGTU * (k + 1)], msps)
                if k == 3:
                    nc.sync.dma_start(out=OUTh[b, :, 0:4 * TGTU],
                                      in_=s["osl"][:, 0:4 * TGTU])
                elif k == NBLK_D - 1:
                    nc.sync.dma_start(out=OUTh[b, :, 4 * TGTU:T_GRID],
                                      in_=s["osl"][:, 4 * TGTU:T_GRID])

            # ---------------- emission ----------------
            dec_units = [(b, k) for b in range(BLOC)
                         for k in range(NBLK_D)]
            du = [0]

            def emit_dec(nu=1):
                # deprioritized: dec K-tiles should fill conv-phase gaps, not
                # get hoisted ahead of the encoder where they stall on DMAs
                for _ in range(nu):
                    if du[0] < len(dec_units):
                        b, k = dec_units[du[0]]
                        with tc.high_priority(offset=-100000):
                            dec_block(b, k)
                        du[0] += 1

            for k in range(NBLK_E):
                enc_block(0, k)
                if k == 8:
                    epilogue(0, "A")
            epilogue(0, "B")
            for k in range(NBLK_E):
                enc_block(1, k)
                if k == 8:
                    epilogue(1, "A")
                    stack_shift(C1S, 32, 4, 0)
                    stack_shift(C1S, 32, 4, 1)
            epilogue(1, "B")
            stack_shift(C1S, 32, 4, 2)
            stack_shift(C1S, 32, 4, 3)

            nexts = {0: (F2, 32, 32), 2: (F4, 32, 32)}
            for l in range(4):
                for n in range(4):
                    conv_chunk(l, n)
                    if l < 3 and n >= 1:
                        if l == 1:
                            shift_f3(n - 1)
                        else:
                            stack_shift(*nexts[l][:2], nexts[l][2], n - 1)
                    emit_dec(1)
                    if l == 3 and n == 1:
                        fchain(0, 0)
                        fchain(1, 0)
                        for k in range(3):
                            dec_mm(0, k)
                            dec_mm(1, k)
                if l < 3:
                    if l == 1:
                        shift_f3(3)
                    else:
                        stack_shift(*nexts[l][:2], nexts[l][2], 3)
            emit_dec(len(dec_units))    # drain any remainder

            fchain(0, 1)
            fchain(1, 1)
            for k in range(3, NBLK_D):
                dec_mm(0, k)
                dec_mm(1, k)

    nc.compile()
    return nc


def _hi_lo(vals):
    """Split into f16-exact hi (multiples of 1/16) + small f16 lo."""
    f16, f64 = np.float16, np.float64
    hi = (np.round(np.asarray(vals, f64) * 16.0) / 16.0).astype(f16)
    lo = (np.asarray(vals, f64) - hi.astype(f64)).astype(f16)
    return hi, lo


def make_inmaps(inputs):
    f32 = np.float32
    f16 = np.float16
    f64 = np.float64
    xc = np.asarray(inputs["xc"])[..., 0].astype(f32)
    yc = np.asarray(inputs["yc"])[..., 0].astype(f32)
    xt = np.asarray(inputs["xt"])[..., 0].astype(f32)
    ls_psi = f64(np.float32(inputs["ls_psi"]))
    os_psi = f64(np.float32(inputs["os_psi"]))
    ls_rho = f64(np.float32(inputs["ls_rho"]))
    os_rho = f64(np.float32(inputs["os_rho"]))
    w = [np.asarray(inputs[f"w{i}"]).astype(f32) for i in (1, 2, 3, 4)]
    bs = [np.asarray(inputs[f"b{i}"]).astype(f32) for i in (1, 2, 3, 4)]

    lower = np.minimum(xc.min(), xt.min())
    upper = np.maximum(xc.max(), xt.max())
    t64 = np.linspace(f64(lower), f64(upper), T_GRID)
    delta = (t64[-1] - t64[0]) / (T_GRID - 1)

    a_psi = -0.5 / (ls_psi * ls_psi)
    a_rho = -0.5 / (ls_rho * ls_rho)
    m_psi = np.sqrt(ETH / -a_psi)
    m_rho = np.sqrt(ETH / -a_rho)

    perm_c = np.argsort(xc, axis=1, kind="stable")
    xcs = np.take_along_axis(xc, perm_c, 1).astype(f64)
    ycs = np.take_along_axis(yc, perm_c, 1).astype(f64)
    perm_t = np.argsort(xt, axis=1, kind="stable")
    xts = np.take_along_axis(xt, perm_t, 1).astype(f64)

    # encoder windows (16 blocks of 128 grid points)
    eidx = np.zeros((B, NBLK_E, 2), np.int64)
    for k in range(NBLK_E):
        lo = t64[WBLK_E * k] - m_psi
        hi = t64[WBLK_E * (k + 1) - 1] + m_psi
        for b in range(B):
            eidx[b, k, 0] = np.searchsorted(xcs[b], lo)
            eidx[b, k, 1] = np.searchsorted(xcs[b], hi)
    ecnt = eidx[:, :, 1] - eidx[:, :, 0]
    NCH_E = [max(1, int(np.ceil(ecnt[:, k].max() / 128)))
             for k in range(NBLK_E)]
    assert max(NCH_E) <= 4, NCH_E

    # decoder grid-chunk windows per xt quantile-block
    J0S, J1S = [], []
    for k in range(NBLK_D):
        xmin = min(xts[b, TGTU * k] for b in range(B))
        xmax = max(xts[b, TGTU * (k + 1) - 1] for b in range(B))
        g0 = max(0, int(np.searchsorted(t64, xmin - m_rho)) - 1)
        g1 = min(T_GRID - 1, int(np.searchsorted(t64, xmax + m_rho)))
        J0S.append(g0 // 128)
        J1S.append(g1 // 128 + 1)
    NCH_D = [J1S[k] - J0S[k] for k in range(NBLK_D)]
    assert max(NCH_D) <= 4, NCH_D
    SE = sum(NCH_E)
    MAXNE = max(NCH_E)
    RE = 2 + 5 * MAXNE
    BW = MAXNE * WBLK_E
    KGWC = BW + 2 * NBLK_D * 128
    XBC = NBLK_E * 128 + NBLK_D * 2 * TGTU

    tpr = (np.arange(WBLK_E) - (WBLK_E - 1) / 2.0) * delta
    te2_hi, te2_lo = _hi_lo(a_psi * tpr * tpr)
    th_hi, th_lo = _hi_lo(tpr)

    # KGW: [BDE | TDB0 | TDB1]
    KGW = np.zeros((17, KGWC), f16)
    for c in range(MAXNE):
        sl = slice(WBLK_E * c, WBLK_E * (c + 1))
        KGW[0, sl] = te2_hi
        KGW[1, sl] = te2_lo
        KGW[2 + 5 * c, sl] = 1
        KGW[3 + 5 * c, sl] = 1
        KGW[4 + 5 * c, sl] = th_hi
        KGW[5 + 5 * c, sl] = th_lo
        KGW[6 + 5 * c, sl] = th_hi
    for k in range(NBLK_D):
        gv = t64[128 * J0S[k]:128 * J1S[k]]
        cb = (gv[0] + gv[-1]) / 2.0
        tv = gv - cb
        for half in range(2):
            o = BW + 1024 * half
            ksl = slice(o + 128 * k, o + 128 * (k + 1))
            KGW[0:2, ksl] = 1
        for c in range(NCH_D[k]):
            half, cc = divmod(c, 2)
            o = BW + 1024 * half
            ksl = slice(o + 128 * k, o + 128 * (k + 1))
            tvc = tv[128 * c:128 * (c + 1)]
            gb_hi, gb_lo = _hi_lo(a_rho * tvc * tvc)
            v_hi, v_lo = _hi_lo(-2.0 * a_rho * tvc)
            KGW[2 + 5 * cc, ksl] = gb_hi
            KGW[3 + 5 * cc, ksl] = gb_lo
            KGW[4 + 5 * cc, ksl] = v_hi
            KGW[5 + 5 * cc, ksl] = v_hi
            KGW[6 + 5 * cc, ksl] = v_lo

    # conv1 t channel: affine in t -> 2 static rows + bias + edge fix
    t_hi, t_lo = _hi_lo(t64)
    TROW = np.stack([t_hi, t_lo], 0)
    A1 = w[0][:, 0, :].astype(f64).sum(1)
    C1 = bs[0].astype(f64) + delta * (w[0][:, 0, :].astype(f64)
                                      * (np.arange(5) - 2)).sum(1)
    L, U = t64[0], t64[-1]
    CR = np.zeros((32, 4), f64)
    w10 = w[0][:, 0, :].astype(f64)
    for half in range(2):
        r = slice(16 * half, 16 * half + 16)
        CR[r, 0] = -w10[:, 0] * (L - 2 * delta) - w10[:, 1] * (L - delta)
        CR[r, 1] = -w10[:, 0] * (L - delta)
        CR[r, 2] = -w10[:, 4] * (U + delta)
        CR[r, 3] = -w10[:, 3] * (U + delta) - w10[:, 4] * (U + 2 * delta)

    # block-diagonal batched conv weights, packed into WALL [128, 332]
    W1n = np.zeros((128, 32), f16)
    W1n[4, :] = np.tile(A1.astype(f16), 2)
    W1n[5, :] = np.tile(A1.astype(f16), 2)
    for o in range(4):
        for half in range(2):
            W1n[32 * o + 2 * half, 16 * half:16 * half + 16] = \
                w[0][:, 1, o].astype(f16)
            W1n[32 * o + 1 + 2 * half, 16 * half:16 * half + 16] = \
                w[0][:, 2, o].astype(f16)
    W1e = np.zeros((4, 32), f16)
    for half in range(2):
        W1e[2 * half, 16 * half:16 * half + 16] = w[0][:, 1, 4].astype(f16)
        W1e[1 + 2 * half, 16 * half:16 * half + 16] = w[0][:, 2, 4].astype(f16)
    W2n = np.zeros((128, 64), f16)
    for o in range(4):
        for half in range(2):
            W2n[32 * o + 16 * half:32 * o + 16 * half + 16,
                32 * half:32 * half + 32] = w[1][:, :, o].T.astype(f16)
    W2e = np.zeros((32, 64), f16)
    for half in range(2):
        W2e[16 * half:16 * half + 16, 32 * half:32 * half + 32] = \
            w[1][:, :, 4].T.astype(f16)
    W3n = np.zeros((128, 32), f16)
    for o in range(2):
        for half in range(2):
            W3n[64 * o + 32 * half:64 * o + 32 * half + 32,
                16 * half:16 * half + 16] = w[2][:, :, o].T.astype(f16)
    W3b = np.zeros((128, 32), f16)
    for o in (2, 3):
        for half in range(2):
            W3b[64 * (o - 2) + 32 * half:64 * (o - 2) + 32 * half + 32,
                16 * half:16 * half + 16] = w[2][:, :, o].T.astype(f16)
    W3e4 = np.zeros((64, 32), f16)
    for half in range(2):
        W3e4[32 * half:32 * half + 32, 16 * half:16 * half + 16] = \
            w[2][:, :, 4].T.astype(f16)
    W4n = np.zeros((128, 4), f16)
    for o in range(4):
        for half in range(2):
            W4n[32 * o + 16 * half:32 * o + 16 * half + 16,
                2 * half:2 * half + 2] = w[3][:, :, o].T.astype(f16)
    W4e = np.zeros((32, 4), f16)
    for half in range(2):
        W4e[16 * half:16 * half + 16, 2 * half:2 * half + 2] = \
            w[3][:, :, 4].T.astype(f16)
    WALL = np.zeros((128, 332), f16)
    WALL[0:128, 0:32] = W1n
    WALL[0:4, 32:64] = W1e
    WALL[0:128, 64:128] = W2n
    WALL[0:32, 128:192] = W2e
    WALL[0:128, 192:224] = W3n
    WALL[0:128, 224:256] = W3b
    WALL[0:64, 256:288] = W3e4
    WALL[0:128, 320:324] = W4n
    WALL[0:32, 324:328] = W4e
    WALL[0:4, 328:332] = np.eye(4, dtype=f16)

    BALL = np.zeros((64, 8), f32)
    BALL[0:64, 0] = np.concatenate([bs[1], bs[1]])
    BALL[0:32, 1] = np.concatenate([bs[2], bs[2]])
    BALL[0:32, 2] = np.concatenate([C1, C1]).astype(f32)
    BALL[0:32, 3:7] = CR.astype(f32)

    shared = {"KGW": KGW, "WALL": WALL, "BALL": BALL, "TROW": TROW}

    in_maps = []
    for core in range(NCORES):
        m = dict(shared)
        XB = np.zeros((BLOC, 17, XBC), f16)
        PHI = np.zeros((BLOC, 128, 2 * SE), f16)
        for bb in range(BLOC):
            b = core * BLOC + bb
            XB[bb, 0:2, 0:NBLK_E * 128] = 1
            base = 0
            for k in range(NBLK_E):
                ck = (t64[WBLK_E * k] + t64[WBLK_E * (k + 1) - 1]) / 2.0
                i0, i1 = eidx[b, k]
                nv = int(i1 - i0)
                ns = 128 * NCH_E[k]
                xv = np.zeros(ns, f64)
                xv[:nv] = xcs[b, i0:i1] - ck
                bias = np.full(ns, -60.0, f64)
                bias[:nv] = a_psi * xv[:nv] * xv[:nv]
                uv = np.zeros(ns, f64)
                uv[:nv] = -2.0 * a_psi * xv[:nv]
                ph = np.zeros((ns, 2), f64)
                ph[:nv, 0] = os_psi
                ph[:nv, 1] = os_psi * ycs[b, i0:i1]
                ksl = slice(128 * k, 128 * (k + 1))
                for c in range(NCH_E[k]):
                    sl = slice(128 * c, 128 * (c + 1))
                    b_hi, b_lo = _hi_lo(bias[sl])
                    u_hi, u_lo = _hi_lo(uv[sl])
                    XB[bb, 2 + 5 * c, ksl] = b_hi
                    XB[bb, 3 + 5 * c, ksl] = b_lo
                    XB[bb, 4 + 5 * c, ksl] = u_hi
                    XB[bb, 5 + 5 * c, ksl] = u_hi
                    XB[bb, 6 + 5 * c, ksl] = u_lo
                    PHI[bb, :, 2 * (base + c)] = ph[sl, 0].astype(f16)
                    PHI[bb, :, 2 * (base + c) + 1] = ph[sl, 1].astype(f16)
                base += NCH_E[k]
            for k in range(NBLK_D):
                gv = t64[128 * J0S[k]:128 * J1S[k]]
                cb = (gv[0] + gv[-1]) / 2.0
                i0, i1 = TGTU * k, TGTU * (k + 1)
                assert xts[b, i0] - m_rho >= gv[0] - delta or J0S[k] == 0
                assert xts[b, i1 - 1] + m_rho <= gv[-1] + delta \
                    or J1S[k] == 16
                xv = xts[b, i0:i1] - cb
                xb_hi, xb_lo = _hi_lo(a_rho * xv * xv)
                x_hi, x_lo = _hi_lo(xv)
                k0 = NBLK_E * 128 + 2 * TGTU * k
                for cc in range(2):
                    csl = slice(k0 + TGTU * cc, k0 + TGTU * (cc + 1))
                    XB[bb, 0, csl] = xb_hi
                    XB[bb, 1, csl] = xb_lo
                    XB[bb, 2 + 5 * cc, csl] = 1
                    XB[bb, 3 + 5 * cc, csl] = 1
                    XB[bb, 4 + 5 * cc, csl] = x_hi
                    XB[bb, 5 + 5 * cc, csl] = x_lo
                    XB[bb, 6 + 5 * cc, csl] = x_hi
        m["XB"] = XB
        m["PHI"] = PHI
        in_maps.append(m)

    cfg = {
        "NCH_E": NCH_E, "NCH_D": NCH_D, "J0S": J0S,
        "os_rho": float(os_rho), "b4_0": float(bs[3][0]),
        "b4_1": float(bs[3][1]),
    }
    aux = {"perm_t": perm_t}
    return in_maps, cfg, aux


def kernel(**inputs):
    from concourse.bass_utils import run_bass_kernel_spmd

    in_maps, cfg, aux = make_inmaps(inputs)
    key = (tuple(cfg["NCH_E"]), tuple(cfg["NCH_D"]), tuple(cfg["J0S"]),
           cfg["os_rho"], cfg["b4_0"], cfg["b4_1"])
    if key not in _PROG_CACHE:
        _PROG_CACHE[key] = build_program(cfg)
    nc = _PROG_CACHE[key]

    res = run_bass_kernel_spmd(nc, in_maps, core_ids=list(range(NCORES)))
    outs = [np.asarray(res.results[i]["out"]) for i in range(NCORES)]
    packed = np.concatenate(outs, 0)  # [B, 2, N] in sorted-xt order
    out = np.zeros((B, N, 2), np.float32)
    for b in range(B):
        out[b, aux["perm_t"][b], 0] = packed[b, 0]
        out[b, aux["perm_t"][b], 1] = packed[b, 1]
    return out


# revision 66
# speedup vs baseline: 1.6951x; 1.6951x over previous
"""ConvCNP1d Trainium2 kernel, v4.

Banded RBF via host-side sorting (ls = ln2 over a 128-unit range means
entries vanish beyond |d| ~ 2.7; output is un-sorted on the host).

Key structure (see v2/v3 history in git-less comments):
- RBF exponents a*(x-t)^2 are built entirely by one PE matmul per tile
  from hi/lo-split fp16 rank-1 rows (squared terms + cross term), then a
  single fused Exp emits the fp16 K tile.  No per-chunk DVE work.
- Encoder runs on 16 value-blocks of 128 grid points (narrow bands =>
  fewer padded (xc, t) pairs, and [128, <=512B] PSUM tiles so the eps
  pool can quadruple-buffer).  Decoder runs on 8 xt quantile-blocks of
  256 targets against fixed grid chunks.
- Conv decoder is batch-fused (block-diagonal weights process both
  per-core batches in one matmul) with taps folded into the partition
  dim via shifted stack copies at 32-aligned partition bases; tap 4 is
  a second matmul reading the base block at a column offset.  conv1's
  t channel is affine in the grid index: two static hi/lo t rows + a
  bias + an exact 4-column edge correction added into PSUM.
- h0/h1 epilogue folds h into [8, 256] tiles (DMA gather) so the
  reciprocal/ratio run wide, then DMA scatters into the conv1 stack.
- DMA descriptor generation on the sync engine (~0.6us per dma_start)
  is a hidden serializer: inputs are packed into 6 loads split across
  the two HWDGE queues (sync + scalar), outputs accumulate into one
  [2, 2048] tile per batch and leave in one DMA each.
"""

import numpy as np

T_GRID = 2048
B = 16
N = 2048
NCORES = 8
BLOC = B // NCORES
NBLK_E = 16
WBLK_E = T_GRID // NBLK_E   # 128
NBLK_D = 8
TGTU = T_GRID // NBLK_D     # 256
ETH = 7.5                   # exponent cutoff; entries below e^-ETH dropped
RD = 12                     # decoder kgen rows (2 + 5*2 per half)
TP = T_GRID + 8             # padded stack width (data at col j+4-o)

_PROG_CACHE = {}


def build_program(cfg):
    import concourse.bacc as bacc
    import concourse.tile as tile
    from concourse import mybir

    f32 = mybir.dt.float32
    f16 = mybir.dt.float16
    AF = mybir.ActivationFunctionType
    OP = mybir.AluOpType

    NCH_E = cfg["NCH_E"]
    NCH_D = cfg["NCH_D"]
    J0S = cfg["J0S"]
    os_rho = cfg["os_rho"]
    b4_0 = cfg["b4_0"]
    b4_1 = cfg["b4_1"]
    SE = sum(NCH_E)
    MAXNE = max(NCH_E)
    MAXND = max(NCH_D)
    RE = 2 + 5 * MAXNE
    BW = MAXNE * WBLK_E                      # BDE col width
    KGWC = BW + 2 * NBLK_D * 128             # KGW cols
    XBC = NBLK_E * 128 + NBLK_D * 2 * TGTU   # XB cols
    assert MAXNE * WBLK_E <= 512 and MAXND <= 4

    nc = bacc.Bacc(None, target_bir_lowering=False)

    KGWh = nc.declare_dram_parameter("KGW", [17, KGWC], f16, isOutput=False)
    XBh = nc.declare_dram_parameter("XB", [BLOC, 17, XBC], f16, isOutput=False)
    PHIh = nc.declare_dram_parameter("PHI", [BLOC, 128, 2 * SE], f16, isOutput=False)
    WALLh = nc.declare_dram_parameter("WALL", [128, 332], f16, isOutput=False)
    BALLh = nc.declare_dram_parameter("BALL", [64, 8], f32, isOutput=False)
    TROWh = nc.declare_dram_parameter("TROW", [2, T_GRID], f16, isOutput=False)
    OUTh = nc.declare_dram_parameter("out", [BLOC, 2, T_GRID], f32, isOutput=True)

    with tile.TileContext(nc) as tc:
        with (
            tc.tile_pool(name="singles", bufs=1) as singles,
            tc.tile_pool(name="perb", bufs=2) as perb,
            tc.tile_pool(name="kpool", bufs=4) as kpool,
            tc.tile_pool(name="k2keep", bufs=1) as k2keep,
            tc.tile_pool(name="small", bufs=1) as small,
            tc.tile_pool(name="psE", bufs=2, space="PSUM") as psE,
            tc.tile_pool(name="psD", bufs=2, space="PSUM") as psD,
            tc.tile_pool(name="psC", bufs=2, space="PSUM") as psC,
        ):
            # ---- loads: split into queue-parallel pieces; a small first
            # piece covers the critical path (enc block 0 + PHI + TDB) ----
            KGW = singles.tile([17, KGWC], f16)
            st = [dict() for _ in range(BLOC)]
            for b in range(BLOC):
                s = st[b]
                s["XB"] = perb.tile([17, XBC], f16, tag="XB", name="XB_sb")
                s["PHI"] = perb.tile([128, 2 * SE], f16, tag="PHI", name="PHI_sb")
                s["h"] = perb.tile([2, T_GRID], f32, tag="h_sb", name="h_sb")
                for hf, w in (("A", 144), ("B", 112)):
                    s[f"hg0{hf}"] = perb.tile([8, w], f32, tag=f"hg0{hf}",
                                              name=f"hg0{hf}")
                    s[f"hg1{hf}"] = perb.tile([8, w], f32, tag=f"hg1{hf}",
                                              name=f"hg1{hf}")
                    s[f"rec{hf}"] = perb.tile([8, w], f32, tag=f"rec{hf}",
                                              name=f"rec{hf}")
                    s[f"h0f{hf}"] = perb.tile([8, w], f16, tag=f"h0f{hf}",
                                              name=f"h0f{hf}")
                    s[f"ratf{hf}"] = perb.tile([8, w], f16, tag=f"ratf{hf}",
                                               name=f"ratf{hf}")
                s["fT"] = perb.tile([128, 2, 16], f16, tag="fT", name="fT")
                s["osl"] = perb.tile([2, T_GRID], f32, tag="osl", name="osl")
            NE = NBLK_E * 128
            nc.sync.dma_start(out=KGW[0:17, 0:256], in_=KGWh[0:17, 0:256])
            nc.sync.dma_start(out=st[0]["XB"][0:17, 0:256],
                              in_=XBh[0, 0:17, 0:256])
            if BW > 256:
                nc.sync.dma_start(out=KGW[0:17, 256:BW],
                                  in_=KGWh[0:17, 256:BW])
            nc.sync.dma_start(out=st[0]["XB"][0:17, 256:512],
                              in_=XBh[0, 0:17, 256:512])
            nc.sync.dma_start(out=st[0]["PHI"][:, 0:SE], in_=PHIh[0, :, 0:SE])
            nc.sync.dma_start(out=st[0]["PHI"][:, SE:2 * SE],
                              in_=PHIh[0, :, SE:2 * SE])
            for c0 in range(512, NE, 512):
                nc.sync.dma_start(out=st[0]["XB"][0:17, c0:c0 + 512],
                                  in_=XBh[0, 0:17, c0:c0 + 512])
            nc.sync.dma_start(out=KGW[0:12, BW:BW + 1024],
                              in_=KGWh[0:12, BW:BW + 1024])
            nc.sync.dma_start(out=KGW[0:12, BW + 1024:KGWC],
                              in_=KGWh[0:12, BW + 1024:KGWC])
            for c0 in range(NE, XBC, 1024):
                nc.sync.dma_start(out=st[0]["XB"][0:12, c0:c0 + 1024],
                                  in_=XBh[0, 0:12, c0:c0 + 1024])
            # batch-1 inputs via gpsimd (idle early; keeps the scalar queue
            # free for Exps), small singles appended to the sync queue.
            # Each piece is gated on enc(0) block 0's h so the transfers
            # don't contend with the critical-path head DMAs.
            def gated_b1_load(rows, c0):
                nc.vector.tensor_copy(st[1]["XB"][0:1, c0:c0 + 1],
                                      st[0]["h"][0:1, 0:1])
                nc.gpsimd.dma_start(out=st[1]["XB"][0:rows, c0:c0 + 1024],
                                    in_=XBh[1, 0:rows, c0:c0 + 1024])

            for c0 in range(0, NE, 1024):
                gated_b1_load(17, c0)
            for c0 in range(NE, XBC, 1024):
                gated_b1_load(12, c0)
            nc.sync.dma_start(out=st[1]["PHI"][:, 0:SE],
                              in_=PHIh[1, :, 0:SE])
            nc.sync.dma_start(out=st[1]["PHI"][:, SE:2 * SE],
                              in_=PHIh[1, :, SE:2 * SE])
            WALL = singles.tile([128, 332], f16)
            nc.sync.dma_start(out=WALL, in_=WALLh[:, :])
            BALL = singles.tile([64, 8], f32)
            nc.sync.dma_start(out=BALL, in_=BALLh[:, :])

            def bde(rows, c0, c1):
                return KGW[0:rows, c0:c1]

            def tdb(half, rows, k):
                o = BW + 1024 * half
                return KGW[0:rows, o + 128 * k:o + 128 * (k + 1)]

            def xcb(b, rows, k):
                return st[b]["XB"][0:rows, 128 * k:128 * (k + 1)]

            def xtq(b, rows, k, tot):
                o = NBLK_E * 128 + 2 * TGTU * k
                return st[b]["XB"][0:rows, o:o + tot]

            W1n = WALL[0:100, 0:32]
            W1e = WALL[0:4, 32:64]
            W2n = WALL[0:128, 64:128]
            W2e = WALL[0:32, 128:192]
            W3n = WALL[0:128, 192:224]
            W3b = WALL[0:128, 224:256]
            W3e4 = WALL[0:64, 256:288]
            W4n = WALL[0:128, 320:324]
            W4e = WALL[0:32, 324:328]
            ID4 = WALL[0:4, 328:332]
            B2a = BALL[0:64, 0:1]
            B3a = BALL[0:32, 1:2]
            C1a = BALL[0:32, 2:3]
            CRa = BALL[0:32, 3:7]

            # conv stacks (shared by both batches; taps in partition blocks;
            # block 0 rows 0-3 = data so shift copies read from base 0,
            # rows 4-5 of block 0 = the static affine t rows)
            C1S = singles.tile([100, TP], f16)
            nc.vector.memset(C1S, 0.0)
            nc.sync.dma_start(out=C1S[4:6, 2:2 + T_GRID], in_=TROWh[:, :])
            F2 = singles.tile([128, TP], f16)   # 4 taps x (16ch x 2b)
            F3 = singles.tile([128, TP], f16)   # taps 0,1 x (32ch x 2b)
            F3B = singles.tile([128, TP], f16)  # taps 2,3 x (32ch x 2b)
            F4 = singles.tile([128, TP], f16)   # 4 taps x (16ch x 2b)
            for F, blk in ((F2, 32), (F3, 64), (F3B, 64), (F4, 32)):
                for o in range(128 // blk):
                    nc.vector.memset(F[blk * o:blk * o + blk, 0:4], 0.0)
                    nc.vector.memset(F[blk * o:blk * o + blk, T_GRID:TP], 0.0)
            FRAW = singles.tile([4, T_GRID], f16)  # b0mu,b0sg,b1mu,b1sg

            def enc_block(b, k):
                s = st[b]
                nch = NCH_E[k]
                base = sum(NCH_E[:k])
                rows = 2 + 5 * nch
                tot = nch * WBLK_E
                eps = psE.tile([128, BW], f32, tag="E", name="E_ps")
                nc.tensor.matmul(eps[:, 0:tot], xcb(b, rows, k),
                                 bde(rows, 0, tot), start=True, stop=True)
                kt = kpool.tile([128, BW], f16, tag="K", name="K1t")
                nc.scalar.activation(out=kt[:, 0:tot], in_=eps[:, 0:tot],
                                     func=AF.Exp)
                hps = psC.tile([2, TGTU], f32, tag="c", name="h_ps")
                for c in range(nch):
                    nc.tensor.matmul(
                        hps[:, 0:WBLK_E],
                        s["PHI"][:, 2 * (base + c):2 * (base + c) + 2],
                        kt[:, WBLK_E * c:WBLK_E * (c + 1)],
                        start=(c == 0), stop=(c == nch - 1),
                    )
                nc.vector.tensor_copy(
                    s["h"][:, WBLK_E * k:WBLK_E * (k + 1)], hps[:, 0:WBLK_E])

            def dec_block(b, k):
                # both window-halves matmul into one 2-bank PSUM tile so a
                # single fused Exp emits the whole block's K tile
                s = st[b]
                nch = NCH_D[k]
                tot = nch * TGTU
                s[f"k2t_{k}"] = k2keep.tile(
                    [128, MAXND * TGTU], f16, tag=f"k2_{b}_{k}",
                    name=f"k2_{b}_{k}")
                eps = psD.tile([128, MAXND * TGTU], f32, tag="D", name="D_ps")
                # gate: tiny WAW dep on enc(0)'s finished h keeps the sim
                # scheduler from hoisting dec K-tiles ahead of the encoder
                # (where they stall the in-order PE queue on XTQ DMAs)
                nc.vector.tensor_copy(eps[0:2, 0:1],
                                      st[0]["h"][0:2, T_GRID - 1:T_GRID])
                nc.tensor.matmul(eps[:, 0:512], tdb(0, 12, k),
                                 xtq(b, 12, k, 512), start=True, stop=True)
                if tot > 512:
                    rows1 = 2 + 5 * (nch - 2)
                    nc.tensor.matmul(eps[:, 512:tot], tdb(1, rows1, k),
                                     xtq(b, rows1, k, tot - 512),
                                     start=True, stop=True)
                nc.scalar.activation(out=s[f"k2t_{k}"][:, 0:tot],
                                     in_=eps[:, 0:tot], func=AF.Exp)

            def epilogue(b, hf):
                # half-pipelined (A = grid cols 0:1152, B = 1152:2048) so the
                # conv1 stack's first chunks are ready before the encoder
                # finishes; gpsimd-issued DMAs keep descriptor generation off
                # the busy HWDGE queues
                s = st[b]
                o, w = (0, 1152) if hf == "A" else (1152, 896)
                nc.gpsimd.dma_start(out=s[f"hg0{hf}"],
                                    in_=s["h"][0:1, o:o + w])
                nc.gpsimd.dma_start(out=s[f"hg1{hf}"],
                                    in_=s["h"][1:2, o:o + w])
                nc.vector.reciprocal_approx_fast(s[f"rec{hf}"], s[f"hg0{hf}"])
                nc.vector.tensor_mul(s[f"ratf{hf}"], s[f"hg1{hf}"],
                                     s[f"rec{hf}"])
                nc.vector.tensor_copy(s[f"h0f{hf}"], s[f"hg0{hf}"])
                nc.gpsimd.dma_start(
                    out=C1S[2 * b:2 * b + 1, 4 + o:4 + o + w],
                    in_=s[f"h0f{hf}"])
                nc.gpsimd.dma_start(
                    out=C1S[2 * b + 1:2 * b + 2, 4 + o:4 + o + w],
                    in_=s[f"ratf{hf}"])

            def conv_chunk(l, n):
                c0 = 512 * n
                if l == 0:
                    ps = psC.tile([32, 512], f32, tag="c", name="c_ps")
                    nc.tensor.matmul(ps, W1n, C1S[:, 2 + c0:2 + c0 + 512],
                                     start=True, stop=False)
                    nc.tensor.matmul(ps, W1e, C1S[0:4, 6 + c0:6 + c0 + 512],
                                     start=False, stop=True)
                    if n == 0:
                        nc.vector.tensor_add(ps[:, 0:2], ps[:, 0:2], CRa[:, 0:2])
                    if n == 3:
                        nc.vector.tensor_add(ps[:, 510:512], ps[:, 510:512],
                                             CRa[:, 2:4])
                    nc.scalar.activation(out=F2[0:32, 4 + c0:4 + c0 + 512],
                                         in_=ps, func=AF.Relu, bias=C1a)
                elif l == 1:
                    ps = psC.tile([64, 512], f32, tag="c", name="c_ps")
                    nc.tensor.matmul(ps, W2n, F2[:, 2 + c0:2 + c0 + 512],
                                     start=True, stop=False)
                    nc.tensor.matmul(ps, W2e, F2[0:32, 6 + c0:6 + c0 + 512],
                                     start=False, stop=True)
                    nc.scalar.activation(out=F3[0:64, 4 + c0:4 + c0 + 512],
                                         in_=ps, func=AF.Relu, bias=B2a)
                elif l == 2:
                    ps = psC.tile([32, 512], f32, tag="c", name="c_ps")
                    nc.tensor.matmul(ps, W3n, F3[:, 2 + c0:2 + c0 + 512],
                                     start=True, stop=False)
                    nc.tensor.matmul(ps, W3b, F3B[:, 2 + c0:2 + c0 + 512],
                                     start=False, stop=False)
                    nc.tensor.matmul(ps, W3e4,
                                     F3[0:64, 6 + c0:6 + c0 + 512],
                                     start=False, stop=True)
                    nc.scalar.activation(out=F4[0:32, 4 + c0:4 + c0 + 512],
                                         in_=ps, func=AF.Relu, bias=B3a)
                else:
                    ps = psC.tile([4, 512], f32, tag="c", name="c_ps")
                    nc.tensor.matmul(ps, W4n, F4[:, 2 + c0:2 + c0 + 512],
                                     start=True, stop=False)
                    nc.tensor.matmul(ps, W4e, F4[0:32, 6 + c0:6 + c0 + 512],
                                     start=False, stop=True)
                    nc.vector.tensor_copy(FRAW[:, c0:c0 + 512], ps)

            def stack_shift(F, blk, rows, n):
                # per-chunk tap-block shifts (+4 col overlap so block-o reads
                # never need the next chunk's shift) let the next layer start
                # before this layer's later chunks finish
                c0 = 512 * n
                w = min(516, TP - 4 - c0)
                for o in range(1, 128 // blk):
                    nc.vector.tensor_copy(
                        F[blk * o:blk * o + rows, 4 + c0 - o:4 + c0 - o + w],
                        F[0:rows, 4 + c0:4 + c0 + w])

            def shift_f3(n):
                c0 = 512 * n
                w = min(516, TP - 4 - c0)
                src = F3[0:64, 4 + c0:4 + c0 + w]
                nc.vector.tensor_copy(
                    F3[64:128, 3 + c0:3 + c0 + w], src)
                nc.vector.tensor_copy(
                    F3B[0:64, 2 + c0:2 + c0 + w], src)
                nc.vector.tensor_copy(
                    F3B[64:128, 1 + c0:1 + c0 + w], src)

            # softplus via exp + ln1p(u) minimax poly keeps Scalar inside
            # the Exp/Relu act table (no mid-program ACT_TABLE_LOAD)
            LN1P = (-0.055459313742069534, 0.21866548366220714,
                    -0.46644243862756585, 0.9962619482337954,
                    6.944574454161809e-05)
            MUB = singles.tile([128, 16], f32)
            nc.vector.memset(MUB, float(os_rho * b4_0))

            def fchain(b, half):
                # per-half (grid chunks 0:8 / 8:16) so the first dec_mms can
                # overlap conv4's later chunks
                s = st[b]
                j0 = 8 * half
                ftp = psC.tile([128, 32], f16, tag="c", name="ftp")
                for j in range(8):
                    nc.tensor.transpose(
                        ftp[:, 4 * j:4 * j + 4],
                        FRAW[:, 128 * (j0 + j):128 * (j0 + j + 1)],
                        ID4)
                mu = ftp[:, 2 * b::4]
                sg = ftp[:, 2 * b + 1::4]
                fsl = slice(j0, j0 + 8)
                x = small.tile([128, 8], f32, tag="t1", name="t1")
                e = small.tile([128, 8], f32, tag="t2", name="t2")
                p = small.tile([128, 8], f32, tag="t3", name="t3")
                r = small.tile([128, 8], f32, tag="t4", name="t4")
                nc.vector.scalar_tensor_tensor(
                    s["fT"][:, 0, fsl], mu, float(os_rho), MUB[:, 0:8],
                    OP.mult, OP.add)
                nc.vector.tensor_scalar_add(x, sg, float(b4_1))
                nc.vector.scalar_tensor_tensor(e, x, -1.0, x, OP.mult, OP.min)
                nc.scalar.activation(out=e, in_=e, func=AF.Exp)
                # os_rho * ln1p(e) via nested Horner, one STT per step
                cs = [float(os_rho * c) for c in LN1P]
                nc.vector.tensor_scalar_mul(p, e, cs[0])
                for ck in cs[1:-1]:
                    nc.vector.scalar_tensor_tensor(p, p, ck, e,
                                                   OP.add, OP.mult)
                nc.vector.tensor_scalar_add(p, p, cs[-1])
                nc.vector.tensor_scalar(r, x, float(os_rho), 0.0,
                                        OP.mult, OP.max)
                nc.vector.tensor_add(s["fT"][:, 1, fsl], p, r)

            def dec_mm(b, k):
                s = st[b]
                kt = s[f"k2t_{k}"]
                nch = NCH_D[k]
                msps = psD.tile([2, TGTU], f32, tag="D", name="ms_ps")
                for c in range(nch):
                    nc.tensor.matmul(
                        msps,
                        s["fT"][:, :, J0S[k] + c],
                        kt[:, TGTU * c:TGTU * (c + 1)],
                        start=(c == 0), stop=(c == nch - 1),
                    )
                nc.vector.tensor_copy(
                    s["osl"][:, TGTU * k:TGTU * (k + 1)], msps)
                if k == 3:
                    nc.sync.dma_start(out=OUTh[b, :, 0:4 * TGTU],
                                      in_=s["osl"][:, 0:4 * TGTU])
                elif k == NBLK_D - 1:
                    nc.sync.dma_start(out=OUTh[b, :, 4 * TGTU:T_GRID],
                                      in_=s["osl"][:, 4 * TGTU:T_GRID])

            # ---------------- emission ----------------
            dec_units = [(b, k) for b in range(BLOC)
                         for k in range(NBLK_D)]
            du = [0]

            def emit_dec(nu=1):
                # deprioritized: dec K-tiles should fill conv-phase gaps, not
                # get hoisted ahead of the encoder where they stall on DMAs
                for _ in range(nu):
                    if du[0] < len(dec_units):
                        b, k = dec_units[du[0]]
                        with tc.high_priority(offset=-100000):
                            dec_block(b, k)
                        du[0] += 1

            for k in range(NBLK_E):
                enc_block(0, k)
                if k == 8:
                    epilogue(0, "A")
            epilogue(0, "B")
            for k in range(NBLK_E):
                enc_block(1, k)
                if k == 8:
                    epilogue(1, "A")
                    stack_shift(C1S, 32, 4, 0)
                    stack_shift(C1S, 32, 4, 1)
            epilogue(1, "B")
            stack_shift(C1S, 32, 4, 2)
            stack_shift(C1S, 32, 4, 3)

            nexts = {0: (F2, 32, 32), 2: (F4, 32, 32)}
            for l in range(4):
                for n in range(4):
                    conv_chunk(l, n)
                    if l < 3 and n >= 1:
                        if l == 1:
                            shift_f3(n - 1)
                        else:
                            stack_shift(*nexts[l][:2], nexts[l][2], n - 1)
                    emit_dec(1)
                    if l == 3 and n == 1:
                        fchain(0, 0)
                        fchain(1, 0)
                        for k in range(3):
                            dec_mm(0, k)
                            dec_mm(1, k)
                if l < 3:
                    if l == 1:
                        shift_f3(3)
                    else:
                        stack_shift(*nexts[l][:2], nexts[l][2], 3)
            emit_dec(len(dec_units))    # drain any remainder

            fchain(0, 1)
            fchain(1, 1)
            for k in range(3, NBLK_D):
                dec_mm(0, k)
                dec_mm(1, k)

    nc.compile()
    return nc


def _hi_lo(vals):
    """Split into f16-exact hi (multiples of 1/16) + small f16 lo."""
    f16, f64 = np.float16, np.float64
    hi = (np.round(np.asarray(vals, f64) * 16.0) / 16.0).astype(f16)
    lo = (np.asarray(vals, f64) - hi.astype(f64)).astype(f16)
    return hi, lo


def make_inmaps(inputs):
    f32 = np.float32
    f16 = np.float16
    f64 = np.float64
    xc = np.asarray(inputs["xc"])[..., 0].astype(f32)
    yc = np.asarray(inputs["yc"])[..., 0].astype(f32)
    xt = np.asarray(inputs["xt"])[..., 0].astype(f32)
    ls_psi = f64(np.float32(inputs["ls_psi"]))
    os_psi = f64(np.float32(inputs["os_psi"]))
    ls_rho = f64(np.float32(inputs["ls_rho"]))
    os_rho = f64(np.float32(inputs["os_rho"]))
    w = [np.asarray(inputs[f"w{i}"]).astype(f32) for i in (1, 2, 3, 4)]
    bs = [np.asarray(inputs[f"b{i}"]).astype(f32) for i in (1, 2, 3, 4)]

    lower = np.minimum(xc.min(), xt.min())
    upper = np.maximum(xc.max(), xt.max())
    t64 = np.linspace(f64(lower), f64(upper), T_GRID)
    delta = (t64[-1] - t64[0]) / (T_GRID - 1)

    a_psi = -0.5 / (ls_psi * ls_psi)
    a_rho = -0.5 / (ls_rho * ls_rho)
    m_psi = np.sqrt(ETH / -a_psi)
    m_rho = np.sqrt(ETH / -a_rho)

    perm_c = np.argsort(xc, axis=1, kind="stable")
    xcs = np.take_along_axis(xc, perm_c, 1).astype(f64)
    ycs = np.take_along_axis(yc, perm_c, 1).astype(f64)
    perm_t = np.argsort(xt, axis=1, kind="stable")
    xts = np.take_along_axis(xt, perm_t, 1).astype(f64)

    # encoder windows (16 blocks of 128 grid points)
    eidx = np.zeros((B, NBLK_E, 2), np.int64)
    for k in range(NBLK_E):
        lo = t64[WBLK_E * k] - m_psi
        hi = t64[WBLK_E * (k + 1) - 1] + m_psi
        for b in range(B):
            eidx[b, k, 0] = np.searchsorted(xcs[b], lo)
            eidx[b, k, 1] = np.searchsorted(xcs[b], hi)
    ecnt = eidx[:, :, 1] - eidx[:, :, 0]
    NCH_E = [max(1, int(np.ceil(ecnt[:, k].max() / 128)))
             for k in range(NBLK_E)]
    assert max(NCH_E) <= 4, NCH_E

    # decoder grid-chunk windows per xt quantile-block
    J0S, J1S = [], []
    for k in range(NBLK_D):
        xmin = min(xts[b, TGTU * k] for b in range(B))
        xmax = max(xts[b, TGTU * (k + 1) - 1] for b in range(B))
        g0 = max(0, int(np.searchsorted(t64, xmin - m_rho)) - 1)
        g1 = min(T_GRID - 1, int(np.searchsorted(t64, xmax + m_rho)))
        J0S.append(g0 // 128)
        J1S.append(g1 // 128 + 1)
    NCH_D = [J1S[k] - J0S[k] for k in range(NBLK_D)]
    assert max(NCH_D) <= 4, NCH_D
    SE = sum(NCH_E)
    MAXNE = max(NCH_E)
    RE = 2 + 5 * MAXNE
    BW = MAXNE * WBLK_E
    KGWC = BW + 2 * NBLK_D * 128
    XBC = NBLK_E * 128 + NBLK_D * 2 * TGTU

    tpr = (np.arange(WBLK_E) - (WBLK_E - 1) / 2.0) * delta
    te2_hi, te2_lo = _hi_lo(a_psi * tpr * tpr)
    th_hi, th_lo = _hi_lo(tpr)

    # KGW: [BDE | TDB0 | TDB1]
    KGW = np.zeros((17, KGWC), f16)
    for c in range(MAXNE):
        sl = slice(WBLK_E * c, WBLK_E * (c + 1))
        KGW[0, sl] = te2_hi
        KGW[1, sl] = te2_lo
        KGW[2 + 5 * c, sl] = 1
        KGW[3 + 5 * c, sl] = 1
        KGW[4 + 5 * c, sl] = th_hi
        KGW[5 + 5 * c, sl] = th_lo
        KGW[6 + 5 * c, sl] = th_hi
    for k in range(NBLK_D):
        gv = t64[128 * J0S[k]:128 * J1S[k]]
        cb = (gv[0] + gv[-1]) / 2.0
        tv = gv - cb
        for half in range(2):
            o = BW + 1024 * half
            ksl = slice(o + 128 * k, o + 128 * (k + 1))
            KGW[0:2, ksl] = 1
        for c in range(NCH_D[k]):
            half, cc = divmod(c, 2)
            o = BW + 1024 * half
            ksl = slice(o + 128 * k, o + 128 * (k + 1))
            tvc = tv[128 * c:128 * (c + 1)]
            gb_hi, gb_lo = _hi_lo(a_rho * tvc * tvc)
            v_hi, v_lo = _hi_lo(-2.0 * a_rho * tvc)
            KGW[2 + 5 * cc, ksl] = gb_hi
            KGW[3 + 5 * cc, ksl] = gb_lo
            KGW[4 + 5 * cc, ksl] = v_hi
            KGW[5 + 5 * cc, ksl] = v_hi
            KGW[6 + 5 * cc, ksl] = v_lo

    # conv1 t channel: affine in t -> 2 static rows + bias + edge fix
    t_hi, t_lo = _hi_lo(t64)
    TROW = np.stack([t_hi, t_lo], 0)
    A1 = w[0][:, 0, :].astype(f64).sum(1)
    C1 = bs[0].astype(f64) + delta * (w[0][:, 0, :].astype(f64)
                                      * (np.arange(5) - 2)).sum(1)
    L, U = t64[0], t64[-1]
    CR = np.zeros((32, 4), f64)
    w10 = w[0][:, 0, :].astype(f64)
    for half in range(2):
        r = slice(16 * half, 16 * half + 16)
        CR[r, 0] = -w10[:, 0] * (L - 2 * delta) - w10[:, 1] * (L - delta)
        CR[r, 1] = -w10[:, 0] * (L - delta)
        CR[r, 2] = -w10[:, 4] * (U + delta)
        CR[r, 3] = -w10[:, 3] * (U + delta) - w10[:, 4] * (U + 2 * delta)

    # block-diagonal batched conv weights, packed into WALL [128, 332]
    W1n = np.zeros((128, 32), f16)
    W1n[4, :] = np.tile(A1.astype(f16), 2)
    W1n[5, :] = np.tile(A1.astype(f16), 2)
    for o in range(4):
        for half in range(2):
            W1n[32 * o + 2 * half, 16 * half:16 * half + 16] = \
                w[0][:, 1, o].astype(f16)
            W1n[32 * o + 1 + 2 * half, 16 * half:16 * half + 16] = \
                w[0][:, 2, o].astype(f16)
    W1e = np.zeros((4, 32), f16)
    for half in range(2):
        W1e[2 * half, 16 * half:16 * half + 16] = w[0][:, 1, 4].astype(f16)
        W1e[1 + 2 * half, 16 * half:16 * half + 16] = w[0][:, 2, 4].astype(f16)
    W2n = np.zeros((128, 64), f16)
    for o in range(4):
        for half in range(2):
            W2n[32 * o + 16 * half:32 * o + 16 * half + 16,
                32 * half:32 * half + 32] = w[1][:, :, o].T.astype(f16)
    W2e = np.zeros((32, 64), f16)
    for half in range(2):
        W2e[16 * half:16 * half + 16, 32 * half:32 * half + 32] = \
            w[1][:, :, 4].T.astype(f16)
    W3n = np.zeros((128, 32), f16)
    for o in range(2):
        for half in range(2):
            W3n[64 * o + 32 * half:64 * o + 32 * half + 32,
                16 * half:16 * half + 16] = w[2][:, :, o].T.astype(f16)
    W3b = np.zeros((128, 32), f16)
    for o in (2, 3):
        for half in range(2):
            W3b[64 * (o - 2) + 32 * half:64 * (o - 2) + 32 * half + 32,
                16 * half:16 * half + 16] = w[2][:, :, o].T.astype(f16)
    W3e4 = np.zeros((64, 32), f16)
    for half in range(2):
        W3e4[32 * half:32 * half + 32, 16 * half:16 * half + 16] = \
            w[2][:, :, 4].T.astype(f16)
    W4n = np.zeros((128, 4), f16)
    for o in range(4):
        for half in range(2):
            W4n[32 * o + 16 * half:32 * o + 16 * half + 16,
                2 * half:2 * half + 2] = w[3][:, :, o].T.astype(f16)
    W4e = np.zeros((32, 4), f16)
    for half in range(2):
        W4e[16 * half:16 * half + 16, 2 * half:2 * half + 2] = \
            w[3][:, :, 4].T.astype(f16)
    WALL = np.zeros((128, 332), f16)
    WALL[0:128, 0:32] = W1n
    WALL[0:4, 32:64] = W1e
    WALL[0:128, 64:128] = W2n
    WALL[0:32, 128:192] = W2e
    WALL[0:128, 192:224] = W3n
    WALL[0:128, 224:256] = W3b
    WALL[0:64, 256:288] = W3e4
    WALL[0:128, 320:324] = W4n
    WALL[0:32, 324:328] = W4e
    WALL[0:4, 328:332] = np.eye(4, dtype=f16)

    BALL = np.zeros((64, 8), f32)
    BALL[0:64, 0] = np.concatenate([bs[1], bs[1]])
    BALL[0:32, 1] = np.concatenate([bs[2], bs[2]])
    BALL[0:32, 2] = np.concatenate([C1, C1]).astype(f32)
    BALL[0:32, 3:7] = CR.astype(f32)

    shared = {"KGW": KGW, "WALL": WALL, "BALL": BALL, "TROW": TROW}

    in_maps = []
    for core in range(NCORES):
        m = dict(shared)
        XB = np.zeros((BLOC, 17, XBC), f16)
        PHI = np.zeros((BLOC, 128, 2 * SE), f16)
        for bb in range(BLOC):
            b = core * BLOC + bb
            XB[bb, 0:2, 0:NBLK_E * 128] = 1
            base = 0
            for k in range(NBLK_E):
                ck = (t64[WBLK_E * k] + t64[WBLK_E * (k + 1) - 1]) / 2.0
                i0, i1 = eidx[b, k]
                nv = int(i1 - i0)
                ns = 128 * NCH_E[k]
                xv = np.zeros(ns, f64)
                xv[:nv] = xcs[b, i0:i1] - ck
                bias = np.full(ns, -60.0, f64)
                bias[:nv] = a_psi * xv[:nv] * xv[:nv]
                uv = np.zeros(ns, f64)
                uv[:nv] = -2.0 * a_psi * xv[:nv]
                ph = np.zeros((ns, 2), f64)
                ph[:nv, 0] = os_psi
                ph[:nv, 1] = os_psi * ycs[b, i0:i1]
                ksl = slice(128 * k, 128 * (k + 1))
                for c in range(NCH_E[k]):
                    sl = slice(128 * c, 128 * (c + 1))
                    b_hi, b_lo = _hi_lo(bias[sl])
                    u_hi, u_lo = _hi_lo(uv[sl])
                    XB[bb, 2 + 5 * c, ksl] = b_hi
                    XB[bb, 3 + 5 * c, ksl] = b_lo
                    XB[bb, 4 + 5 * c, ksl] = u_hi
                    XB[bb, 5 + 5 * c, ksl] = u_hi
                    XB[bb, 6 + 5 * c, ksl] = u_lo
                    PHI[bb, :, 2 * (base + c)] = ph[sl, 0].astype(f16)
                    PHI[bb, :, 2 * (base + c) + 1] = ph[sl, 1].astype(f16)
                base += NCH_E[k]
            for k in range(NBLK_D):
                gv = t64[128 * J0S[k]:128 * J1S[k]]
                cb = (gv[0] + gv[-1]) / 2.0
                i0, i1 = TGTU * k, TGTU * (k + 1)
                assert xts[b, i0] - m_rho >= gv[0] - delta or J0S[k] == 0
                assert xts[b, i1 - 1] + m_rho <= gv[-1] + delta \
                    or J1S[k] == 16
                xv = xts[b, i0:i1] - cb
                xb_hi, xb_lo = _hi_lo(a_rho * xv * xv)
                x_hi, x_lo = _hi_lo(xv)
                k0 = NBLK_E * 128 + 2 * TGTU * k
                for cc in range(2):
                    csl = slice(k0 + TGTU * cc, k0 + TGTU * (cc + 1))
                    XB[bb, 0, csl] = xb_hi
                    XB[bb, 1, csl] = xb_lo
                    XB[bb, 2 + 5 * cc, csl] = 1
                    XB[bb, 3 + 5 * cc, csl] = 1
                    XB[bb, 4 + 5 * cc, csl] = x_hi
                    XB[bb, 5 + 5 * cc, csl] = x_lo
                    XB[bb, 6 + 5 * cc, csl] = x_hi
        m["XB"] = XB
        m["PHI"] = PHI
        in_maps.append(m)

    cfg = {
        "NCH_E": NCH_E, "NCH_D": NCH_D, "J0S": J0S,
        "os_rho": float(os_rho), "b4_0": float(bs[3][0]),
        "b4_1": float(bs[3][1]),
    }
    aux = {"perm_t": perm_t}
    return in_maps, cfg, aux


def kernel(**inputs):
    from concourse.bass_utils import run_bass_kernel_spmd

    in_maps, cfg, aux = make_inmaps(inputs)
    key = (tuple(cfg["NCH_E"]), tuple(cfg["NCH_D"]), tuple(cfg["J0S"]),
           cfg["os_rho"], cfg["b4_0"], cfg["b4_1"])
    if key not in _PROG_CACHE:
        _PROG_CACHE[key] = build_program(cfg)
    nc = _PROG_CACHE[key]

    res = run_bass_kernel_spmd(nc, in_maps, core_ids=list(range(NCORES)))
    outs = [np.asarray(res.results[i]["out"]) for i in range(NCORES)]
    packed = np.concatenate(outs, 0)  # [B, 2, N] in sorted-xt order
    out = np.zeros((B, N, 2), np.float32)
    for b in range(B):
        out[b, aux["perm_t"][b], 0] = packed[b, 0]
        out[b, aux["perm_t"][b], 1] = packed[b, 1]
    return out


# revision 67
# speedup vs baseline: 1.7412x; 1.0272x over previous
"""ConvCNP1d Trainium2 kernel, v4.

Banded RBF via host-side sorting (ls = ln2 over a 128-unit range means
entries vanish beyond |d| ~ 2.7; output is un-sorted on the host).

Key structure (see v2/v3 history in git-less comments):
- RBF exponents a*(x-t)^2 are built entirely by one PE matmul per tile
  from hi/lo-split fp16 rank-1 rows (squared terms + cross term), then a
  single fused Exp emits the fp16 K tile.  No per-chunk DVE work.
- Encoder runs on 16 value-blocks of 128 grid points (narrow bands =>
  fewer padded (xc, t) pairs, and [128, <=512B] PSUM tiles so the eps
  pool can quadruple-buffer).  Decoder runs on 8 xt quantile-blocks of
  256 targets against fixed grid chunks.
- Conv decoder is batch-fused (block-diagonal weights process both
  per-core batches in one matmul) with taps folded into the partition
  dim via shifted stack copies at 32-aligned partition bases; tap 4 is
  a second matmul reading the base block at a column offset.  conv1's
  t channel is affine in the grid index: two static hi/lo t rows + a
  bias + an exact 4-column edge correction added into PSUM.
- h0/h1 epilogue folds h into [8, 256] tiles (DMA gather) so the
  reciprocal/ratio run wide, then DMA scatters into the conv1 stack.
- DMA descriptor generation on the sync engine (~0.6us per dma_start)
  is a hidden serializer: inputs are packed into 6 loads split across
  the two HWDGE queues (sync + scalar), outputs accumulate into one
  [2, 2048] tile per batch and leave in one DMA each.
"""

import numpy as np

T_GRID = 2048
B = 16
N = 2048
NCORES = 8
BLOC = B // NCORES
NBLK_E = 16
WBLK_E = T_GRID // NBLK_E   # 128
NBLK_D = 8
TGTU = T_GRID // NBLK_D     # 256
ETH = 7.5                   # exponent cutoff; entries below e^-ETH dropped
RD = 12                     # decoder kgen rows (2 + 5*2 per half)
TP = T_GRID + 8             # padded stack width (data at col j+4-o)

_PROG_CACHE = {}


def build_program(cfg):
    import concourse.bacc as bacc
    import concourse.tile as tile
    from concourse import mybir

    f32 = mybir.dt.float32
    f16 = mybir.dt.float16
    AF = mybir.ActivationFunctionType
    OP = mybir.AluOpType

    NCH_E = cfg["NCH_E"]
    NCH_D = cfg["NCH_D"]
    J0S = cfg["J0S"]
    os_rho = cfg["os_rho"]
    b4_0 = cfg["b4_0"]
    b4_1 = cfg["b4_1"]
    SE = sum(NCH_E)
    MAXNE = max(NCH_E)
    MAXND = max(NCH_D)
    RE = 2 + 5 * MAXNE
    BW = MAXNE * WBLK_E                      # BDE col width
    KGWC = BW + 2 * NBLK_D * 128             # KGW cols
    XBC = NBLK_E * 128 + NBLK_D * 2 * TGTU   # XB cols
    assert MAXNE * WBLK_E <= 512 and MAXND <= 4

    nc = bacc.Bacc(None, target_bir_lowering=False)

    KGWh = nc.declare_dram_parameter("KGW", [17, KGWC], f16, isOutput=False)
    XBh = nc.declare_dram_parameter("XB", [BLOC, 17, XBC], f16, isOutput=False)
    PHIh = nc.declare_dram_parameter("PHI", [BLOC, 128, 2 * SE], f16, isOutput=False)
    WALLh = nc.declare_dram_parameter("WALL", [128, 332], f16, isOutput=False)
    BALLh = nc.declare_dram_parameter("BALL", [64, 8], f32, isOutput=False)
    TROWh = nc.declare_dram_parameter("TROW", [2, T_GRID], f16, isOutput=False)
    OUTh = nc.declare_dram_parameter("out", [BLOC, 2, T_GRID], f32, isOutput=True)

    with tile.TileContext(nc) as tc:
        with (
            tc.tile_pool(name="singles", bufs=1) as singles,
            tc.tile_pool(name="perb", bufs=2) as perb,
            tc.tile_pool(name="kpool", bufs=4) as kpool,
            tc.tile_pool(name="k2keep", bufs=1) as k2keep,
            tc.tile_pool(name="small", bufs=1) as small,
            tc.tile_pool(name="psE", bufs=2, space="PSUM") as psE,
            tc.tile_pool(name="psD", bufs=2, space="PSUM") as psD,
            tc.tile_pool(name="psC", bufs=2, space="PSUM") as psC,
        ):
            # ---- loads: split into queue-parallel pieces; a small first
            # piece covers the critical path (enc block 0 + PHI + TDB) ----
            KGW = singles.tile([17, KGWC], f16)
            st = [dict() for _ in range(BLOC)]
            for b in range(BLOC):
                s = st[b]
                s["XB"] = perb.tile([17, XBC], f16, tag="XB", name="XB_sb")
                s["PHI"] = perb.tile([128, 2 * SE], f16, tag="PHI", name="PHI_sb")
                s["h"] = perb.tile([2, T_GRID], f32, tag="h_sb", name="h_sb")
                for hf, w in (("A", 144), ("B", 112)):
                    s[f"hg0{hf}"] = perb.tile([8, w], f32, tag=f"hg0{hf}",
                                              name=f"hg0{hf}")
                    s[f"hg1{hf}"] = perb.tile([8, w], f32, tag=f"hg1{hf}",
                                              name=f"hg1{hf}")
                    s[f"rec{hf}"] = perb.tile([8, w], f32, tag=f"rec{hf}",
                                              name=f"rec{hf}")
                    s[f"h0f{hf}"] = perb.tile([8, w], f16, tag=f"h0f{hf}",
                                              name=f"h0f{hf}")
                    s[f"ratf{hf}"] = perb.tile([8, w], f16, tag=f"ratf{hf}",
                                               name=f"ratf{hf}")
                s["fT"] = perb.tile([128, 2, 16], f16, tag="fT", name="fT")
                s["osl"] = perb.tile([2, T_GRID], f32, tag="osl", name="osl")
            NE = NBLK_E * 128
            nc.sync.dma_start(out=KGW[0:17, 0:256], in_=KGWh[0:17, 0:256])
            nc.sync.dma_start(out=st[0]["XB"][0:17, 0:256],
                              in_=XBh[0, 0:17, 0:256])
            if BW > 256:
                nc.sync.dma_start(out=KGW[0:17, 256:BW],
                                  in_=KGWh[0:17, 256:BW])
            nc.sync.dma_start(out=st[0]["XB"][0:17, 256:512],
                              in_=XBh[0, 0:17, 256:512])
            nc.sync.dma_start(out=st[0]["PHI"][:, 0:SE], in_=PHIh[0, :, 0:SE])
            nc.sync.dma_start(out=st[0]["PHI"][:, SE:2 * SE],
                              in_=PHIh[0, :, SE:2 * SE])
            for c0 in range(512, NE, 512):
                nc.sync.dma_start(out=st[0]["XB"][0:17, c0:c0 + 512],
                                  in_=XBh[0, 0:17, c0:c0 + 512])
            nc.sync.dma_start(out=KGW[0:12, BW:BW + 1024],
                              in_=KGWh[0:12, BW:BW + 1024])
            nc.sync.dma_start(out=KGW[0:12, BW + 1024:KGWC],
                              in_=KGWh[0:12, BW + 1024:KGWC])
            for c0 in range(NE, XBC, 1024):
                nc.sync.dma_start(out=st[0]["XB"][0:12, c0:c0 + 1024],
                                  in_=XBh[0, 0:12, c0:c0 + 1024])
            # batch-1 inputs via gpsimd (idle early; keeps the scalar queue
            # free for Exps), small singles appended to the sync queue.
            # Each piece is gated on enc(0) block 0's h so the transfers
            # don't contend with the critical-path head DMAs.
            def gated_b1_load(rows, c0):
                nc.vector.tensor_copy(st[1]["XB"][0:1, c0:c0 + 1],
                                      st[0]["h"][0:1, 0:1])
                nc.gpsimd.dma_start(out=st[1]["XB"][0:rows, c0:c0 + 1024],
                                    in_=XBh[1, 0:rows, c0:c0 + 1024])

            for c0 in range(0, NE, 1024):
                gated_b1_load(17, c0)
            for c0 in range(NE, XBC, 1024):
                gated_b1_load(12, c0)
            nc.sync.dma_start(out=st[1]["PHI"][:, 0:SE],
                              in_=PHIh[1, :, 0:SE])
            nc.sync.dma_start(out=st[1]["PHI"][:, SE:2 * SE],
                              in_=PHIh[1, :, SE:2 * SE])
            WALL = singles.tile([128, 332], f16)
            nc.sync.dma_start(out=WALL, in_=WALLh[:, :])
            BALL = singles.tile([64, 8], f32)
            nc.sync.dma_start(out=BALL, in_=BALLh[:, :])

            def bde(rows, c0, c1):
                return KGW[0:rows, c0:c1]

            def tdb(half, rows, k):
                o = BW + 1024 * half
                return KGW[0:rows, o + 128 * k:o + 128 * (k + 1)]

            def xcb(b, rows, k):
                return st[b]["XB"][0:rows, 128 * k:128 * (k + 1)]

            def xtq(b, rows, k, tot):
                o = NBLK_E * 128 + 2 * TGTU * k
                return st[b]["XB"][0:rows, o:o + tot]

            W1n = WALL[0:100, 0:32]
            W1e = WALL[0:4, 32:64]
            W2n = WALL[0:128, 64:128]
            W2e = WALL[0:32, 128:192]
            W3n = WALL[0:128, 192:224]
            W3b = WALL[0:128, 224:256]
            W3e4 = WALL[0:64, 256:288]
            W4n = WALL[0:128, 320:324]
            W4e = WALL[0:32, 324:328]
            ID4 = WALL[0:4, 328:332]
            B2a = BALL[0:64, 0:1]
            B3a = BALL[0:32, 1:2]
            C1a = BALL[0:32, 2:3]
            CRa = BALL[0:32, 3:7]

            # conv stacks (shared by both batches; taps in partition blocks;
            # block 0 rows 0-3 = data so shift copies read from base 0,
            # rows 4-5 of block 0 = the static affine t rows)
            C1S = singles.tile([100, TP], f16)
            nc.vector.memset(C1S, 0.0)
            nc.sync.dma_start(out=C1S[4:6, 2:2 + T_GRID], in_=TROWh[:, :])
            F2 = singles.tile([128, TP], f16)   # 4 taps x (16ch x 2b)
            F3 = singles.tile([128, TP], f16)   # taps 0,1 x (32ch x 2b)
            F3B = singles.tile([128, TP], f16)  # taps 2,3 x (32ch x 2b)
            F4 = singles.tile([128, TP], f16)   # 4 taps x (16ch x 2b)
            for F, blk in ((F2, 32), (F3, 64), (F3B, 64), (F4, 32)):
                for o in range(128 // blk):
                    nc.vector.memset(F[blk * o:blk * o + blk, 0:4], 0.0)
                    nc.vector.memset(F[blk * o:blk * o + blk, T_GRID:TP], 0.0)
            FRAW = singles.tile([4, T_GRID], f16)  # b0mu,b0sg,b1mu,b1sg

            def enc_block(b, k):
                s = st[b]
                nch = NCH_E[k]
                base = sum(NCH_E[:k])
                rows = 2 + 5 * nch
                tot = nch * WBLK_E
                eps = psE.tile([128, BW], f32, tag="E", name="E_ps")
                nc.tensor.matmul(eps[:, 0:tot], xcb(b, rows, k),
                                 bde(rows, 0, tot), start=True, stop=True)
                kt = kpool.tile([128, BW], f16, tag="K", name="K1t")
                nc.scalar.activation(out=kt[:, 0:tot], in_=eps[:, 0:tot],
                                     func=AF.Exp)
                hps = psC.tile([2, TGTU], f32, tag="c", name="h_ps")
                for c in range(nch):
                    nc.tensor.matmul(
                        hps[:, 0:WBLK_E],
                        s["PHI"][:, 2 * (base + c):2 * (base + c) + 2],
                        kt[:, WBLK_E * c:WBLK_E * (c + 1)],
                        start=(c == 0), stop=(c == nch - 1),
                    )
                nc.vector.tensor_copy(
                    s["h"][:, WBLK_E * k:WBLK_E * (k + 1)], hps[:, 0:WBLK_E])

            def dec_block(b, k):
                # both window-halves matmul into one 2-bank PSUM tile so a
                # single fused Exp emits the whole block's K tile
                s = st[b]
                nch = NCH_D[k]
                tot = nch * TGTU
                s[f"k2t_{k}"] = k2keep.tile(
                    [128, MAXND * TGTU], f16, tag=f"k2_{b}_{k}",
                    name=f"k2_{b}_{k}")
                eps = psD.tile([128, MAXND * TGTU], f32, tag="D", name="D_ps")
                # gate: tiny WAW dep on enc(0)'s finished h keeps the sim
                # scheduler from hoisting dec K-tiles ahead of the encoder
                # (where they stall the in-order PE queue on XTQ DMAs)
                nc.vector.tensor_copy(eps[0:2, 0:1],
                                      st[0]["h"][0:2, T_GRID - 1:T_GRID])
                nc.tensor.matmul(eps[:, 0:512], tdb(0, 12, k),
                                 xtq(b, 12, k, 512), start=True, stop=True)
                if tot > 512:
                    rows1 = 2 + 5 * (nch - 2)
                    nc.tensor.matmul(eps[:, 512:tot], tdb(1, rows1, k),
                                     xtq(b, rows1, k, tot - 512),
                                     start=True, stop=True)
                nc.scalar.activation(out=s[f"k2t_{k}"][:, 0:tot],
                                     in_=eps[:, 0:tot], func=AF.Exp)

            def epilogue(b, hf):
                # half-pipelined (A = grid cols 0:1152, B = 1152:2048) so the
                # conv1 stack's first chunks are ready before the encoder
                # finishes; gpsimd-issued DMAs keep descriptor generation off
                # the busy HWDGE queues
                s = st[b]
                o, w = (0, 1152) if hf == "A" else (1152, 896)
                nc.gpsimd.dma_start(out=s[f"hg0{hf}"],
                                    in_=s["h"][0:1, o:o + w])
                nc.gpsimd.dma_start(out=s[f"hg1{hf}"],
                                    in_=s["h"][1:2, o:o + w])
                nc.vector.reciprocal_approx_fast(s[f"rec{hf}"], s[f"hg0{hf}"])
                nc.vector.tensor_mul(s[f"ratf{hf}"], s[f"hg1{hf}"],
                                     s[f"rec{hf}"])
                nc.vector.tensor_copy(s[f"h0f{hf}"], s[f"hg0{hf}"])
                nc.gpsimd.dma_start(
                    out=C1S[2 * b:2 * b + 1, 4 + o:4 + o + w],
                    in_=s[f"h0f{hf}"])
                nc.gpsimd.dma_start(
                    out=C1S[2 * b + 1:2 * b + 2, 4 + o:4 + o + w],
                    in_=s[f"ratf{hf}"])

            def conv_chunk(l, n):
                c0 = 512 * n
                if l == 0:
                    ps = psC.tile([32, 512], f32, tag="c", name="c_ps")
                    nc.tensor.matmul(ps, W1n, C1S[:, 2 + c0:2 + c0 + 512],
                                     start=True, stop=False)
                    nc.tensor.matmul(ps, W1e, C1S[0:4, 6 + c0:6 + c0 + 512],
                                     start=False, stop=True)
                    if n == 0:
                        nc.vector.tensor_add(ps[:, 0:2], ps[:, 0:2], CRa[:, 0:2])
                    if n == 3:
                        nc.vector.tensor_add(ps[:, 510:512], ps[:, 510:512],
                                             CRa[:, 2:4])
                    nc.scalar.activation(out=F2[0:32, 4 + c0:4 + c0 + 512],
                                         in_=ps, func=AF.Relu, bias=C1a)
                elif l == 1:
                    ps = psC.tile([64, 512], f32, tag="c", name="c_ps")
                    nc.tensor.matmul(ps, W2n, F2[:, 2 + c0:2 + c0 + 512],
                                     start=True, stop=False)
                    nc.tensor.matmul(ps, W2e, F2[0:32, 6 + c0:6 + c0 + 512],
                                     start=False, stop=True)
                    nc.scalar.activation(out=F3[0:64, 4 + c0:4 + c0 + 512],
                                         in_=ps, func=AF.Relu, bias=B2a)
                elif l == 2:
                    ps = psC.tile([32, 512], f32, tag="c", name="c_ps")
                    nc.tensor.matmul(ps, W3n, F3[:, 2 + c0:2 + c0 + 512],
                                     start=True, stop=False)
                    nc.tensor.matmul(ps, W3b, F3B[:, 2 + c0:2 + c0 + 512],
                                     start=False, stop=False)
                    nc.tensor.matmul(ps, W3e4,
                                     F3[0:64, 6 + c0:6 + c0 + 512],
                                     start=False, stop=True)
                    nc.scalar.activation(out=F4[0:32, 4 + c0:4 + c0 + 512],
                                         in_=ps, func=AF.Relu, bias=B3a)
                else:
                    ps = psC.tile([4, 512], f32, tag="c", name="c_ps")
                    nc.tensor.matmul(ps, W4n, F4[:, 2 + c0:2 + c0 + 512],
                                     start=True, stop=False)
                    nc.tensor.matmul(ps, W4e, F4[0:32, 6 + c0:6 + c0 + 512],
                                     start=False, stop=True)
                    nc.vector.tensor_copy(FRAW[:, c0:c0 + 512], ps)

            def stack_shift(F, blk, rows, n):
                # per-chunk tap-block shifts (+4 col overlap so block-o reads
                # never need the next chunk's shift) let the next layer start
                # before this layer's later chunks finish
                c0 = 512 * n
                w = min(516, TP - 4 - c0)
                for o in range(1, 128 // blk):
                    nc.vector.tensor_copy(
                        F[blk * o:blk * o + rows, 4 + c0 - o:4 + c0 - o + w],
                        F[0:rows, 4 + c0:4 + c0 + w])

            def shift_f3(n):
                c0 = 512 * n
                w = min(516, TP - 4 - c0)
                src = F3[0:64, 4 + c0:4 + c0 + w]
                nc.vector.tensor_copy(
                    F3[64:128, 3 + c0:3 + c0 + w], src)
                nc.vector.tensor_copy(
                    F3B[0:64, 2 + c0:2 + c0 + w], src)
                nc.vector.tensor_copy(
                    F3B[64:128, 1 + c0:1 + c0 + w], src)

            # softplus via exp + ln1p(u) minimax poly keeps Scalar inside
            # the Exp/Relu act table (no mid-program ACT_TABLE_LOAD)
            LN1P = (-0.055459313742069534, 0.21866548366220714,
                    -0.46644243862756585, 0.9962619482337954,
                    6.944574454161809e-05)
            MUB = singles.tile([128, 16], f32)
            nc.vector.memset(MUB, float(os_rho * b4_0))

            def fchain(b, half):
                # per-half (grid chunks 0:8 / 8:16) so the first dec_mms can
                # overlap conv4's later chunks
                s = st[b]
                j0 = 8 * half
                ftp = psC.tile([128, 32], f16, tag="c", name="ftp")
                for j in range(8):
                    nc.tensor.transpose(
                        ftp[:, 4 * j:4 * j + 4],
                        FRAW[:, 128 * (j0 + j):128 * (j0 + j + 1)],
                        ID4)
                mu = ftp[:, 2 * b::4]
                sg = ftp[:, 2 * b + 1::4]
                fsl = slice(j0, j0 + 8)
                x = small.tile([128, 8], f32, tag="t1", name="t1")
                e = small.tile([128, 8], f32, tag="t2", name="t2")
                p = small.tile([128, 8], f32, tag="t3", name="t3")
                r = small.tile([128, 8], f32, tag="t4", name="t4")
                nc.vector.scalar_tensor_tensor(
                    s["fT"][:, 0, fsl], mu, float(os_rho), MUB[:, 0:8],
                    OP.mult, OP.add)
                nc.vector.tensor_scalar_add(x, sg, float(b4_1))
                nc.vector.scalar_tensor_tensor(e, x, -1.0, x, OP.mult, OP.min)
                nc.scalar.activation(out=e, in_=e, func=AF.Exp)
                # os_rho * ln1p(e) via nested Horner, one STT per step
                cs = [float(os_rho * c) for c in LN1P]
                nc.vector.tensor_scalar_mul(p, e, cs[0])
                for ck in cs[1:-1]:
                    nc.vector.scalar_tensor_tensor(p, p, ck, e,
                                                   OP.add, OP.mult)
                nc.vector.tensor_scalar_add(p, p, cs[-1])
                nc.vector.tensor_scalar(r, x, float(os_rho), 0.0,
                                        OP.mult, OP.max)
                nc.vector.tensor_add(s["fT"][:, 1, fsl], p, r)

            def dec_mm(b, k):
                s = st[b]
                kt = s[f"k2t_{k}"]
                nch = NCH_D[k]
                # alternate pools: psE is idle by now, so dec_mm gets an
                # effective 4-deep accumulator rotation
                if (2 * b + k) % 2 == 0:
                    msps = psD.tile([2, TGTU], f32, tag="D", name="ms_ps")
                else:
                    msps = psE.tile([2, TGTU], f32, tag="E", name="ms_ps")
                for c in range(nch):
                    nc.tensor.matmul(
                        msps,
                        s["fT"][:, :, J0S[k] + c],
                        kt[:, TGTU * c:TGTU * (c + 1)],
                        start=(c == 0), stop=(c == nch - 1),
                    )
                nc.vector.tensor_copy(
                    s["osl"][:, TGTU * k:TGTU * (k + 1)], msps)
                if k == 3:
                    nc.sync.dma_start(out=OUTh[b, :, 0:4 * TGTU],
                                      in_=s["osl"][:, 0:4 * TGTU])
                elif k == NBLK_D - 1:
                    nc.sync.dma_start(out=OUTh[b, :, 4 * TGTU:T_GRID],
                                      in_=s["osl"][:, 4 * TGTU:T_GRID])

            # ---------------- emission ----------------
            dec_units = [(b, k) for b in range(BLOC)
                         for k in range(NBLK_D)]
            du = [0]

            def emit_dec(nu=1):
                # deprioritized: dec K-tiles should fill conv-phase gaps, not
                # get hoisted ahead of the encoder where they stall on DMAs
                for _ in range(nu):
                    if du[0] < len(dec_units):
                        b, k = dec_units[du[0]]
                        with tc.high_priority(offset=-100000):
                            dec_block(b, k)
                        du[0] += 1

            for k in range(NBLK_E):
                enc_block(0, k)
                if k == 8:
                    epilogue(0, "A")
            epilogue(0, "B")
            for k in range(NBLK_E):
                enc_block(1, k)
                if k == 8:
                    epilogue(1, "A")
                    stack_shift(C1S, 32, 4, 0)
                    stack_shift(C1S, 32, 4, 1)
            epilogue(1, "B")
            stack_shift(C1S, 32, 4, 2)
            stack_shift(C1S, 32, 4, 3)

            nexts = {0: (F2, 32, 32), 2: (F4, 32, 32)}
            for l in range(4):
                for n in range(4):
                    conv_chunk(l, n)
                    if l < 3 and n >= 1:
                        if l == 1:
                            shift_f3(n - 1)
                        else:
                            stack_shift(*nexts[l][:2], nexts[l][2], n - 1)
                    emit_dec(1)
                    if l == 3 and n == 1:
                        fchain(0, 0)
                        fchain(1, 0)
                        for k in range(3):
                            dec_mm(0, k)
                            dec_mm(1, k)
                if l < 3:
                    if l == 1:
                        shift_f3(3)
                    else:
                        stack_shift(*nexts[l][:2], nexts[l][2], 3)
            emit_dec(len(dec_units))    # drain any remainder

            fchain(0, 1)
            fchain(1, 1)
            for k in range(3, NBLK_D):
                dec_mm(0, k)
                dec_mm(1, k)

    nc.compile()
    return nc


def _hi_lo(vals):
    """Split into f16-exact hi (multiples of 1/16) + small f16 lo."""
    f16, f64 = np.float16, np.float64
    hi = (np.round(np.asarray(vals, f64) * 16.0) / 16.0).astype(f16)
    lo = (np.asarray(vals, f64) - hi.astype(f64)).astype(f16)
    return hi, lo


def make_inmaps(inputs):
    f32 = np.float32
    f16 = np.float16
    f64 = np.float64
    xc = np.asarray(inputs["xc"])[..., 0].astype(f32)
    yc = np.asarray(inputs["yc"])[..., 0].astype(f32)
    xt = np.asarray(inputs["xt"])[..., 0].astype(f32)
    ls_psi = f64(np.float32(inputs["ls_psi"]))
    os_psi = f64(np.float32(inputs["os_psi"]))
    ls_rho = f64(np.float32(inputs["ls_rho"]))
    os_rho = f64(np.float32(inputs["os_rho"]))
    w = [np.asarray(inputs[f"w{i}"]).astype(f32) for i in (1, 2, 3, 4)]
    bs = [np.asarray(inputs[f"b{i}"]).astype(f32) for i in (1, 2, 3, 4)]

    lower = np.minimum(xc.min(), xt.min())
    upper = np.maximum(xc.max(), xt.max())
    t64 = np.linspace(f64(lower), f64(upper), T_GRID)
    delta = (t64[-1] - t64[0]) / (T_GRID - 1)

    a_psi = -0.5 / (ls_psi * ls_psi)
    a_rho = -0.5 / (ls_rho * ls_rho)
    m_psi = np.sqrt(ETH / -a_psi)
    m_rho = np.sqrt(ETH / -a_rho)

    perm_c = np.argsort(xc, axis=1, kind="stable")
    xcs = np.take_along_axis(xc, perm_c, 1).astype(f64)
    ycs = np.take_along_axis(yc, perm_c, 1).astype(f64)
    perm_t = np.argsort(xt, axis=1, kind="stable")
    xts = np.take_along_axis(xt, perm_t, 1).astype(f64)

    # encoder windows (16 blocks of 128 grid points)
    eidx = np.zeros((B, NBLK_E, 2), np.int64)
    for k in range(NBLK_E):
        lo = t64[WBLK_E * k] - m_psi
        hi = t64[WBLK_E * (k + 1) - 1] + m_psi
        for b in range(B):
            eidx[b, k, 0] = np.searchsorted(xcs[b], lo)
            eidx[b, k, 1] = np.searchsorted(xcs[b], hi)
    ecnt = eidx[:, :, 1] - eidx[:, :, 0]
    NCH_E = [max(1, int(np.ceil(ecnt[:, k].max() / 128)))
             for k in range(NBLK_E)]
    assert max(NCH_E) <= 4, NCH_E

    # decoder grid-chunk windows per xt quantile-block
    J0S, J1S = [], []
    for k in range(NBLK_D):
        xmin = min(xts[b, TGTU * k] for b in range(B))
        xmax = max(xts[b, TGTU * (k + 1) - 1] for b in range(B))
        g0 = max(0, int(np.searchsorted(t64, xmin - m_rho)) - 1)
        g1 = min(T_GRID - 1, int(np.searchsorted(t64, xmax + m_rho)))
        J0S.append(g0 // 128)
        J1S.append(g1 // 128 + 1)
    NCH_D = [J1S[k] - J0S[k] for k in range(NBLK_D)]
    assert max(NCH_D) <= 4, NCH_D
    SE = sum(NCH_E)
    MAXNE = max(NCH_E)
    RE = 2 + 5 * MAXNE
    BW = MAXNE * WBLK_E
    KGWC = BW + 2 * NBLK_D * 128
    XBC = NBLK_E * 128 + NBLK_D * 2 * TGTU

    tpr = (np.arange(WBLK_E) - (WBLK_E - 1) / 2.0) * delta
    te2_hi, te2_lo = _hi_lo(a_psi * tpr * tpr)
    th_hi, th_lo = _hi_lo(tpr)

    # KGW: [BDE | TDB0 | TDB1]
    KGW = np.zeros((17, KGWC), f16)
    for c in range(MAXNE):
        sl = slice(WBLK_E * c, WBLK_E * (c + 1))
        KGW[0, sl] = te2_hi
        KGW[1, sl] = te2_lo
        KGW[2 + 5 * c, sl] = 1
        KGW[3 + 5 * c, sl] = 1
        KGW[4 + 5 * c, sl] = th_hi
        KGW[5 + 5 * c, sl] = th_lo
        KGW[6 + 5 * c, sl] = th_hi
    for k in range(NBLK_D):
        gv = t64[128 * J0S[k]:128 * J1S[k]]
        cb = (gv[0] + gv[-1]) / 2.0
        tv = gv - cb
        for half in range(2):
            o = BW + 1024 * half
            ksl = slice(o + 128 * k, o + 128 * (k + 1))
            KGW[0:2, ksl] = 1
        for c in range(NCH_D[k]):
            half, cc = divmod(c, 2)
            o = BW + 1024 * half
            ksl = slice(o + 128 * k, o + 128 * (k + 1))
            tvc = tv[128 * c:128 * (c + 1)]
            gb_hi, gb_lo = _hi_lo(a_rho * tvc * tvc)
            v_hi, v_lo = _hi_lo(-2.0 * a_rho * tvc)
            KGW[2 + 5 * cc, ksl] = gb_hi
            KGW[3 + 5 * cc, ksl] = gb_lo
            KGW[4 + 5 * cc, ksl] = v_hi
            KGW[5 + 5 * cc, ksl] = v_hi
            KGW[6 + 5 * cc, ksl] = v_lo

    # conv1 t channel: affine in t -> 2 static rows + bias + edge fix
    t_hi, t_lo = _hi_lo(t64)
    TROW = np.stack([t_hi, t_lo], 0)
    A1 = w[0][:, 0, :].astype(f64).sum(1)
    C1 = bs[0].astype(f64) + delta * (w[0][:, 0, :].astype(f64)
                                      * (np.arange(5) - 2)).sum(1)
    L, U = t64[0], t64[-1]
    CR = np.zeros((32, 4), f64)
    w10 = w[0][:, 0, :].astype(f64)
    for half in range(2):
        r = slice(16 * half, 16 * half + 16)
        CR[r, 0] = -w10[:, 0] * (L - 2 * delta) - w10[:, 1] * (L - delta)
        CR[r, 1] = -w10[:, 0] * (L - delta)
        CR[r, 2] = -w10[:, 4] * (U + delta)
        CR[r, 3] = -w10[:, 3] * (U + delta) - w10[:, 4] * (U + 2 * delta)

    # block-diagonal batched conv weights, packed into WALL [128, 332]
    W1n = np.zeros((128, 32), f16)
    W1n[4, :] = np.tile(A1.astype(f16), 2)
    W1n[5, :] = np.tile(A1.astype(f16), 2)
    for o in range(4):
        for half in range(2):
            W1n[32 * o + 2 * half, 16 * half:16 * half + 16] = \
                w[0][:, 1, o].astype(f16)
            W1n[32 * o + 1 + 2 * half, 16 * half:16 * half + 16] = \
                w[0][:, 2, o].astype(f16)
    W1e = np.zeros((4, 32), f16)
    for half in range(2):
        W1e[2 * half, 16 * half:16 * half + 16] = w[0][:, 1, 4].astype(f16)
        W1e[1 + 2 * half, 16 * half:16 * half + 16] = w[0][:, 2, 4].astype(f16)
    W2n = np.zeros((128, 64), f16)
    for o in range(4):
        for half in range(2):
            W2n[32 * o + 16 * half:32 * o + 16 * half + 16,
                32 * half:32 * half + 32] = w[1][:, :, o].T.astype(f16)
    W2e = np.zeros((32, 64), f16)
    for half in range(2):
        W2e[16 * half:16 * half + 16, 32 * half:32 * half + 32] = \
            w[1][:, :, 4].T.astype(f16)
    W3n = np.zeros((128, 32), f16)
    for o in range(2):
        for half in range(2):
            W3n[64 * o + 32 * half:64 * o + 32 * half + 32,
                16 * half:16 * half + 16] = w[2][:, :, o].T.astype(f16)
    W3b = np.zeros((128, 32), f16)
    for o in (2, 3):
        for half in range(2):
            W3b[64 * (o - 2) + 32 * half:64 * (o - 2) + 32 * half + 32,
                16 * half:16 * half + 16] = w[2][:, :, o].T.astype(f16)
    W3e4 = np.zeros((64, 32), f16)
    for half in range(2):
        W3e4[32 * half:32 * half + 32, 16 * half:16 * half + 16] = \
            w[2][:, :, 4].T.astype(f16)
    W4n = np.zeros((128, 4), f16)
    for o in range(4):
        for half in range(2):
            W4n[32 * o + 16 * half:32 * o + 16 * half + 16,
                2 * half:2 * half + 2] = w[3][:, :, o].T.astype(f16)
    W4e = np.zeros((32, 4), f16)
    for half in range(2):
        W4e[16 * half:16 * half + 16, 2 * half:2 * half + 2] = \
            w[3][:, :, 4].T.astype(f16)
    WALL = np.zeros((128, 332), f16)
    WALL[0:128, 0:32] = W1n
    WALL[0:4, 32:64] = W1e
    WALL[0:128, 64:128] = W2n
    WALL[0:32, 128:192] = W2e
    WALL[0:128, 192:224] = W3n
    WALL[0:128, 224:256] = W3b
    WALL[0:64, 256:288] = W3e4
    WALL[0:128, 320:324] = W4n
    WALL[0:32, 324:328] = W4e
    WALL[0:4, 328:332] = np.eye(4, dtype=f16)

    BALL = np.zeros((64, 8), f32)
    BALL[0:64, 0] = np.concatenate([bs[1], bs[1]])
    BALL[0:32, 1] = np.concatenate([bs[2], bs[2]])
    BALL[0:32, 2] = np.concatenate([C1, C1]).astype(f32)
    BALL[0:32, 3:7] = CR.astype(f32)

    shared = {"KGW": KGW, "WALL": WALL, "BALL": BALL, "TROW": TROW}

    in_maps = []
    for core in range(NCORES):
        m = dict(shared)
        XB = np.zeros((BLOC, 17, XBC), f16)
        PHI = np.zeros((BLOC, 128, 2 * SE), f16)
        for bb in range(BLOC):
            b = core * BLOC + bb
            XB[bb, 0:2, 0:NBLK_E * 128] = 1
            base = 0
            for k in range(NBLK_E):
                ck = (t64[WBLK_E * k] + t64[WBLK_E * (k + 1) - 1]) / 2.0
                i0, i1 = eidx[b, k]
                nv = int(i1 - i0)
                ns = 128 * NCH_E[k]
                xv = np.zeros(ns, f64)
                xv[:nv] = xcs[b, i0:i1] - ck
                bias = np.full(ns, -60.0, f64)
                bias[:nv] = a_psi * xv[:nv] * xv[:nv]
                uv = np.zeros(ns, f64)
                uv[:nv] = -2.0 * a_psi * xv[:nv]
                ph = np.zeros((ns, 2), f64)
                ph[:nv, 0] = os_psi
                ph[:nv, 1] = os_psi * ycs[b, i0:i1]
                ksl = slice(128 * k, 128 * (k + 1))
                for c in range(NCH_E[k]):
                    sl = slice(128 * c, 128 * (c + 1))
                    b_hi, b_lo = _hi_lo(bias[sl])
                    u_hi, u_lo = _hi_lo(uv[sl])
                    XB[bb, 2 + 5 * c, ksl] = b_hi
                    XB[bb, 3 + 5 * c, ksl] = b_lo
                    XB[bb, 4 + 5 * c, ksl] = u_hi
                    XB[bb, 5 + 5 * c, ksl] = u_hi
                    XB[bb, 6 + 5 * c, ksl] = u_lo
                    PHI[bb, :, 2 * (base + c)] = ph[sl, 0].astype(f16)
                    PHI[bb, :, 2 * (base + c) + 1] = ph[sl, 1].astype(f16)
                base += NCH_E[k]
            for k in range(NBLK_D):
                gv = t64[128 * J0S[k]:128 * J1S[k]]
                cb = (gv[0] + gv[-1]) / 2.0
                i0, i1 = TGTU * k, TGTU * (k + 1)
                assert xts[b, i0] - m_rho >= gv[0] - delta or J0S[k] == 0
                assert xts[b, i1 - 1] + m_rho <= gv[-1] + delta \
                    or J1S[k] == 16
                xv = xts[b, i0:i1] - cb
                xb_hi, xb_lo = _hi_lo(a_rho * xv * xv)
                x_hi, x_lo = _hi_lo(xv)
                k0 = NBLK_E * 128 + 2 * TGTU * k
                for cc in range(2):
                    csl = slice(k0 + TGTU * cc, k0 + TGTU * (cc + 1))
                    XB[bb, 0, csl] = xb_hi
                    XB[bb, 1, csl] = xb_lo
                    XB[bb, 2 + 5 * cc, csl] = 1
                    XB[bb, 3 + 5 * cc, csl] = 1
                    XB[bb, 4 + 5 * cc, csl] = x_hi
                    XB[bb, 5 + 5 * cc, csl] = x_lo
                    XB[bb, 6 + 5 * cc, csl] = x_hi
        m["XB"] = XB
        m["PHI"] = PHI
        in_maps.append(m)

    cfg = {
        "NCH_E": NCH_E, "NCH_D": NCH_D, "J0S": J0S,
        "os_rho": float(os_rho), "b4_0": float(bs[3][0]),
        "b4_1": float(bs[3][1]),
    }
    aux = {"perm_t": perm_t}
    return in_maps, cfg, aux


def kernel(**inputs):
    from concourse.bass_utils import run_bass_kernel_spmd

    in_maps, cfg, aux = make_inmaps(inputs)
    key = (tuple(cfg["NCH_E"]), tuple(cfg["NCH_D"]), tuple(cfg["J0S"]),
           cfg["os_rho"], cfg["b4_0"], cfg["b4_1"])
    if key not in _PROG_CACHE:
        _PROG_CACHE[key] = build_program(cfg)
    nc = _PROG_CACHE[key]

    res = run_bass_kernel_spmd(nc, in_maps, core_ids=list(range(NCORES)))
    outs = [np.asarray(res.results[i]["out"]) for i in range(NCORES)]
    packed = np.concatenate(outs, 0)  # [B, 2, N] in sorted-xt order
    out = np.zeros((B, N, 2), np.float32)
    for b in range(B):
        out[b, aux["perm_t"][b], 0] = packed[b, 0]
        out[b, aux["perm_t"][b], 1] = packed[b, 1]
    return out


# revision 68
# speedup vs baseline: 1.7815x; 1.0231x over previous
"""ConvCNP1d Trainium2 kernel, v4.

Banded RBF via host-side sorting (ls = ln2 over a 128-unit range means
entries vanish beyond |d| ~ 2.7; output is un-sorted on the host).

Key structure (see v2/v3 history in git-less comments):
- RBF exponents a*(x-t)^2 are built entirely by one PE matmul per tile
  from hi/lo-split fp16 rank-1 rows (squared terms + cross term), then a
  single fused Exp emits the fp16 K tile.  No per-chunk DVE work.
- Encoder runs on 16 value-blocks of 128 grid points (narrow bands =>
  fewer padded (xc, t) pairs, and [128, <=512B] PSUM tiles so the eps
  pool can quadruple-buffer).  Decoder runs on 8 xt quantile-blocks of
  256 targets against fixed grid chunks.
- Conv decoder is batch-fused (block-diagonal weights process both
  per-core batches in one matmul) with taps folded into the partition
  dim via shifted stack copies at 32-aligned partition bases; tap 4 is
  a second matmul reading the base block at a column offset.  conv1's
  t channel is affine in the grid index: two static hi/lo t rows + a
  bias + an exact 4-column edge correction added into PSUM.
- h0/h1 epilogue folds h into [8, 256] tiles (DMA gather) so the
  reciprocal/ratio run wide, then DMA scatters into the conv1 stack.
- DMA descriptor generation on the sync engine (~0.6us per dma_start)
  is a hidden serializer: inputs are packed into 6 loads split across
  the two HWDGE queues (sync + scalar), outputs accumulate into one
  [2, 2048] tile per batch and leave in one DMA each.
"""

import numpy as np

T_GRID = 2048
B = 16
N = 2048
NCORES = 8
BLOC = B // NCORES
NBLK_E = 16
WBLK_E = T_GRID // NBLK_E   # 128
NBLK_D = 8
TGTU = T_GRID // NBLK_D     # 256
ETH = 7.5                   # exponent cutoff; entries below e^-ETH dropped
RD = 12                     # decoder kgen rows (2 + 5*2 per half)
TP = T_GRID + 8             # padded stack width (data at col j+4-o)

_PROG_CACHE = {}


def build_program(cfg):
    import concourse.bacc as bacc
    import concourse.tile as tile
    from concourse import mybir

    f32 = mybir.dt.float32
    f16 = mybir.dt.float16
    AF = mybir.ActivationFunctionType
    OP = mybir.AluOpType

    NCH_E = cfg["NCH_E"]
    NCH_D = cfg["NCH_D"]
    J0S = cfg["J0S"]
    os_rho = cfg["os_rho"]
    b4_0 = cfg["b4_0"]
    b4_1 = cfg["b4_1"]
    SE = sum(NCH_E)
    MAXNE = max(NCH_E)
    MAXND = max(NCH_D)
    RE = 2 + 5 * MAXNE
    BW = MAXNE * WBLK_E                      # BDE col width
    KGWC = BW + 2 * NBLK_D * 128             # KGW cols
    XBC = NBLK_E * 128 + NBLK_D * 2 * TGTU   # XB cols
    assert MAXNE * WBLK_E <= 512 and MAXND <= 4

    nc = bacc.Bacc(None, target_bir_lowering=False)

    KGWh = nc.declare_dram_parameter("KGW", [17, KGWC], f16, isOutput=False)
    XBh = nc.declare_dram_parameter("XB", [BLOC, 17, XBC], f16, isOutput=False)
    PHIh = nc.declare_dram_parameter("PHI", [BLOC, 128, 2 * SE], f16, isOutput=False)
    WALLh = nc.declare_dram_parameter("WALL", [128, 332], f16, isOutput=False)
    BALLh = nc.declare_dram_parameter("BALL", [64, 8], f32, isOutput=False)
    TROWh = nc.declare_dram_parameter("TROW", [2, T_GRID], f16, isOutput=False)
    OUTh = nc.declare_dram_parameter("out", [BLOC, 2, T_GRID], f32, isOutput=True)

    with tile.TileContext(nc) as tc:
        with (
            tc.tile_pool(name="singles", bufs=1) as singles,
            tc.tile_pool(name="perb", bufs=2) as perb,
            tc.tile_pool(name="kpool", bufs=4) as kpool,
            tc.tile_pool(name="k2keep", bufs=1) as k2keep,
            tc.tile_pool(name="small", bufs=1) as small,
            tc.tile_pool(name="psE", bufs=2, space="PSUM") as psE,
            tc.tile_pool(name="psD", bufs=2, space="PSUM") as psD,
            tc.tile_pool(name="psC", bufs=2, space="PSUM") as psC,
        ):
            # ---- loads: split into queue-parallel pieces; a small first
            # piece covers the critical path (enc block 0 + PHI + TDB) ----
            KGW = singles.tile([17, KGWC], f16)
            st = [dict() for _ in range(BLOC)]
            for b in range(BLOC):
                s = st[b]
                s["XB"] = perb.tile([17, XBC], f16, tag="XB", name="XB_sb")
                s["PHI"] = perb.tile([128, 2 * SE], f16, tag="PHI", name="PHI_sb")
                s["h"] = perb.tile([2, T_GRID], f32, tag="h_sb", name="h_sb")
                for hf, w in (("A", 144), ("B", 112)):
                    s[f"hg0{hf}"] = perb.tile([8, w], f32, tag=f"hg0{hf}",
                                              name=f"hg0{hf}")
                    s[f"hg1{hf}"] = perb.tile([8, w], f32, tag=f"hg1{hf}",
                                              name=f"hg1{hf}")
                    s[f"rec{hf}"] = perb.tile([8, w], f32, tag=f"rec{hf}",
                                              name=f"rec{hf}")
                    s[f"h0f{hf}"] = perb.tile([8, w], f16, tag=f"h0f{hf}",
                                              name=f"h0f{hf}")
                    s[f"ratf{hf}"] = perb.tile([8, w], f16, tag=f"ratf{hf}",
                                               name=f"ratf{hf}")
                s["fT"] = perb.tile([128, 2, 16], f16, tag="fT", name="fT")
                s["osl"] = perb.tile([2, T_GRID], f32, tag="osl", name="osl")
            NE = NBLK_E * 128
            nc.sync.dma_start(out=KGW[0:RE, 0:256], in_=KGWh[0:RE, 0:256])
            nc.sync.dma_start(out=st[0]["XB"][0:RE, 0:256],
                              in_=XBh[0, 0:RE, 0:256])
            if BW > 256:
                nc.sync.dma_start(out=KGW[0:RE, 256:BW],
                                  in_=KGWh[0:RE, 256:BW])
            nc.sync.dma_start(out=st[0]["XB"][0:RE, 256:512],
                              in_=XBh[0, 0:RE, 256:512])
            nc.sync.dma_start(out=st[0]["PHI"][:, 0:SE], in_=PHIh[0, :, 0:SE])
            nc.sync.dma_start(out=st[0]["PHI"][:, SE:2 * SE],
                              in_=PHIh[0, :, SE:2 * SE])
            for c0 in range(512, NE, 512):
                nc.sync.dma_start(out=st[0]["XB"][0:RE, c0:c0 + 512],
                                  in_=XBh[0, 0:RE, c0:c0 + 512])
            nc.sync.dma_start(out=KGW[0:12, BW:BW + 1024],
                              in_=KGWh[0:12, BW:BW + 1024])
            nc.sync.dma_start(out=KGW[0:12, BW + 1024:KGWC],
                              in_=KGWh[0:12, BW + 1024:KGWC])
            for c0 in range(NE, XBC, 1024):
                nc.sync.dma_start(out=st[0]["XB"][0:12, c0:c0 + 1024],
                                  in_=XBh[0, 0:12, c0:c0 + 1024])
            # batch-1 inputs via gpsimd (idle early; keeps the scalar queue
            # free for Exps), small singles appended to the sync queue.
            # Each piece is gated on enc(0) block 0's h so the transfers
            # don't contend with the critical-path head DMAs.
            def gated_b1_load(rows, c0):
                nc.vector.tensor_copy(st[1]["XB"][0:1, c0:c0 + 1],
                                      st[0]["h"][0:1, 0:1])
                nc.gpsimd.dma_start(out=st[1]["XB"][0:rows, c0:c0 + 1024],
                                    in_=XBh[1, 0:rows, c0:c0 + 1024])

            for c0 in range(0, NE, 1024):
                gated_b1_load(RE, c0)
            for c0 in range(NE, XBC, 1024):
                gated_b1_load(12, c0)
            nc.sync.dma_start(out=st[1]["PHI"][:, 0:SE],
                              in_=PHIh[1, :, 0:SE])
            nc.sync.dma_start(out=st[1]["PHI"][:, SE:2 * SE],
                              in_=PHIh[1, :, SE:2 * SE])
            WALL = singles.tile([128, 332], f16)
            nc.sync.dma_start(out=WALL, in_=WALLh[:, :])
            BALL = singles.tile([64, 8], f32)
            nc.sync.dma_start(out=BALL, in_=BALLh[:, :])

            def bde(rows, c0, c1):
                return KGW[0:rows, c0:c1]

            def tdb(half, rows, k):
                o = BW + 1024 * half
                return KGW[0:rows, o + 128 * k:o + 128 * (k + 1)]

            def xcb(b, rows, k):
                return st[b]["XB"][0:rows, 128 * k:128 * (k + 1)]

            def xtq(b, rows, k, tot):
                o = NBLK_E * 128 + 2 * TGTU * k
                return st[b]["XB"][0:rows, o:o + tot]

            W1n = WALL[0:100, 0:32]
            W1e = WALL[0:4, 32:64]
            W2n = WALL[0:128, 64:128]
            W2e = WALL[0:32, 128:192]
            W3n = WALL[0:128, 192:224]
            W3b = WALL[0:128, 224:256]
            W3e4 = WALL[0:64, 256:288]
            W4n = WALL[0:128, 320:324]
            W4e = WALL[0:32, 324:328]
            ID4 = WALL[0:4, 328:332]
            B2a = BALL[0:64, 0:1]
            B3a = BALL[0:32, 1:2]
            C1a = BALL[0:32, 2:3]
            CRa = BALL[0:32, 3:7]

            # conv stacks (shared by both batches; taps in partition blocks;
            # block 0 rows 0-3 = data so shift copies read from base 0,
            # rows 4-5 of block 0 = the static affine t rows)
            C1S = singles.tile([100, TP], f16)
            nc.vector.memset(C1S, 0.0)
            nc.sync.dma_start(out=C1S[4:6, 2:2 + T_GRID], in_=TROWh[:, :])
            F2 = singles.tile([128, TP], f16)   # 4 taps x (16ch x 2b)
            F3 = singles.tile([128, TP], f16)   # taps 0,1 x (32ch x 2b)
            F3B = singles.tile([128, TP], f16)  # taps 2,3 x (32ch x 2b)
            F4 = singles.tile([128, TP], f16)   # 4 taps x (16ch x 2b)
            for F, blk in ((F2, 32), (F3, 64), (F3B, 64), (F4, 32)):
                for o in range(128 // blk):
                    nc.vector.memset(F[blk * o:blk * o + blk, 0:4], 0.0)
                    nc.vector.memset(F[blk * o:blk * o + blk, T_GRID:TP], 0.0)
            FRAW = singles.tile([4, T_GRID], f16)  # b0mu,b0sg,b1mu,b1sg

            def enc_block(b, k):
                s = st[b]
                nch = NCH_E[k]
                base = sum(NCH_E[:k])
                rows = 2 + 5 * nch
                tot = nch * WBLK_E
                eps = psE.tile([128, BW], f32, tag="E", name="E_ps")
                nc.tensor.matmul(eps[:, 0:tot], xcb(b, rows, k),
                                 bde(rows, 0, tot), start=True, stop=True)
                kt = kpool.tile([128, BW], f16, tag="K", name="K1t")
                nc.scalar.activation(out=kt[:, 0:tot], in_=eps[:, 0:tot],
                                     func=AF.Exp)
                hps = psC.tile([2, TGTU], f32, tag="c", name="h_ps")
                for c in range(nch):
                    nc.tensor.matmul(
                        hps[:, 0:WBLK_E],
                        s["PHI"][:, 2 * (base + c):2 * (base + c) + 2],
                        kt[:, WBLK_E * c:WBLK_E * (c + 1)],
                        start=(c == 0), stop=(c == nch - 1),
                    )
                nc.vector.tensor_copy(
                    s["h"][:, WBLK_E * k:WBLK_E * (k + 1)], hps[:, 0:WBLK_E])

            def dec_block(b, k):
                # both window-halves matmul into one 2-bank PSUM tile so a
                # single fused Exp emits the whole block's K tile
                s = st[b]
                nch = NCH_D[k]
                tot = nch * TGTU
                s[f"k2t_{k}"] = k2keep.tile(
                    [128, MAXND * TGTU], f16, tag=f"k2_{b}_{k}",
                    name=f"k2_{b}_{k}")
                eps = psD.tile([128, MAXND * TGTU], f32, tag="D", name="D_ps")
                # gate: tiny WAW dep on enc(0)'s finished h keeps the sim
                # scheduler from hoisting dec K-tiles ahead of the encoder
                # (where they stall the in-order PE queue on XTQ DMAs)
                nc.vector.tensor_copy(eps[0:2, 0:1],
                                      st[0]["h"][0:2, T_GRID - 1:T_GRID])
                nc.tensor.matmul(eps[:, 0:512], tdb(0, 12, k),
                                 xtq(b, 12, k, 512), start=True, stop=True)
                if tot > 512:
                    rows1 = 2 + 5 * (nch - 2)
                    nc.tensor.matmul(eps[:, 512:tot], tdb(1, rows1, k),
                                     xtq(b, rows1, k, tot - 512),
                                     start=True, stop=True)
                nc.scalar.activation(out=s[f"k2t_{k}"][:, 0:tot],
                                     in_=eps[:, 0:tot], func=AF.Exp)

            def epilogue(b, hf):
                # half-pipelined (A = grid cols 0:1152, B = 1152:2048) so the
                # conv1 stack's first chunks are ready before the encoder
                # finishes; gpsimd-issued DMAs keep descriptor generation off
                # the busy HWDGE queues
                s = st[b]
                o, w = (0, 1152) if hf == "A" else (1152, 896)
                nc.gpsimd.dma_start(out=s[f"hg0{hf}"],
                                    in_=s["h"][0:1, o:o + w])
                nc.gpsimd.dma_start(out=s[f"hg1{hf}"],
                                    in_=s["h"][1:2, o:o + w])
                nc.vector.reciprocal_approx_fast(s[f"rec{hf}"], s[f"hg0{hf}"])
                nc.vector.tensor_mul(s[f"ratf{hf}"], s[f"hg1{hf}"],
                                     s[f"rec{hf}"])
                nc.vector.tensor_copy(s[f"h0f{hf}"], s[f"hg0{hf}"])
                nc.gpsimd.dma_start(
                    out=C1S[2 * b:2 * b + 1, 4 + o:4 + o + w],
                    in_=s[f"h0f{hf}"])
                nc.gpsimd.dma_start(
                    out=C1S[2 * b + 1:2 * b + 2, 4 + o:4 + o + w],
                    in_=s[f"ratf{hf}"])

            def conv_chunk(l, n):
                c0 = 512 * n
                if l == 0:
                    ps = psC.tile([32, 512], f32, tag="c", name="c_ps")
                    nc.tensor.matmul(ps, W1n, C1S[:, 2 + c0:2 + c0 + 512],
                                     start=True, stop=False)
                    nc.tensor.matmul(ps, W1e, C1S[0:4, 6 + c0:6 + c0 + 512],
                                     start=False, stop=True)
                    if n == 0:
                        nc.vector.tensor_add(ps[:, 0:2], ps[:, 0:2], CRa[:, 0:2])
                    if n == 3:
                        nc.vector.tensor_add(ps[:, 510:512], ps[:, 510:512],
                                             CRa[:, 2:4])
                    nc.scalar.activation(out=F2[0:32, 4 + c0:4 + c0 + 512],
                                         in_=ps, func=AF.Relu, bias=C1a)
                elif l == 1:
                    ps = psC.tile([64, 512], f32, tag="c", name="c_ps")
                    nc.tensor.matmul(ps, W2n, F2[:, 2 + c0:2 + c0 + 512],
                                     start=True, stop=False)
                    nc.tensor.matmul(ps, W2e, F2[0:32, 6 + c0:6 + c0 + 512],
                                     start=False, stop=True)
                    nc.scalar.activation(out=F3[0:64, 4 + c0:4 + c0 + 512],
                                         in_=ps, func=AF.Relu, bias=B2a)
                elif l == 2:
                    ps = psC.tile([32, 512], f32, tag="c", name="c_ps")
                    nc.tensor.matmul(ps, W3n, F3[:, 2 + c0:2 + c0 + 512],
                                     start=True, stop=False)
                    nc.tensor.matmul(ps, W3b, F3B[:, 2 + c0:2 + c0 + 512],
                                     start=False, stop=False)
                    nc.tensor.matmul(ps, W3e4,
                                     F3[0:64, 6 + c0:6 + c0 + 512],
                                     start=False, stop=True)
                    nc.scalar.activation(out=F4[0:32, 4 + c0:4 + c0 + 512],
                                         in_=ps, func=AF.Relu, bias=B3a)
                else:
                    ps = psC.tile([4, 512], f32, tag="c", name="c_ps")
                    nc.tensor.matmul(ps, W4n, F4[:, 2 + c0:2 + c0 + 512],
                                     start=True, stop=False)
                    nc.tensor.matmul(ps, W4e, F4[0:32, 6 + c0:6 + c0 + 512],
                                     start=False, stop=True)
                    nc.vector.tensor_copy(FRAW[:, c0:c0 + 512], ps)

            def stack_shift(F, blk, rows, n):
                # per-chunk tap-block shifts (+4 col overlap so block-o reads
                # never need the next chunk's shift) let the next layer start
                # before this layer's later chunks finish
                c0 = 512 * n
                w = min(516, TP - 4 - c0)
                for o in range(1, 128 // blk):
                    nc.vector.tensor_copy(
                        F[blk * o:blk * o + rows, 4 + c0 - o:4 + c0 - o + w],
                        F[0:rows, 4 + c0:4 + c0 + w])

            def shift_f3(n):
                c0 = 512 * n
                w = min(516, TP - 4 - c0)
                src = F3[0:64, 4 + c0:4 + c0 + w]
                nc.vector.tensor_copy(
                    F3[64:128, 3 + c0:3 + c0 + w], src)
                nc.vector.tensor_copy(
                    F3B[0:64, 2 + c0:2 + c0 + w], src)
                nc.vector.tensor_copy(
                    F3B[64:128, 1 + c0:1 + c0 + w], src)

            # softplus via exp + ln1p(u) minimax poly keeps Scalar inside
            # the Exp/Relu act table (no mid-program ACT_TABLE_LOAD)
            LN1P = (-0.055459313742069534, 0.21866548366220714,
                    -0.46644243862756585, 0.9962619482337954,
                    6.944574454161809e-05)
            MUB = singles.tile([128, 16], f32)
            nc.vector.memset(MUB, float(os_rho * b4_0))

            def fchain(b, half):
                # per-half (grid chunks 0:8 / 8:16) so the first dec_mms can
                # overlap conv4's later chunks
                s = st[b]
                j0 = 8 * half
                ftp = psC.tile([128, 32], f16, tag="c", name="ftp")
                for j in range(8):
                    nc.tensor.transpose(
                        ftp[:, 4 * j:4 * j + 4],
                        FRAW[:, 128 * (j0 + j):128 * (j0 + j + 1)],
                        ID4)
                mu = ftp[:, 2 * b::4]
                sg = ftp[:, 2 * b + 1::4]
                fsl = slice(j0, j0 + 8)
                x = small.tile([128, 8], f32, tag="t1", name="t1")
                e = small.tile([128, 8], f32, tag="t2", name="t2")
                p = small.tile([128, 8], f32, tag="t3", name="t3")
                r = small.tile([128, 8], f32, tag="t4", name="t4")
                nc.vector.scalar_tensor_tensor(
                    s["fT"][:, 0, fsl], mu, float(os_rho), MUB[:, 0:8],
                    OP.mult, OP.add)
                nc.vector.tensor_scalar_add(x, sg, float(b4_1))
                nc.vector.scalar_tensor_tensor(e, x, -1.0, x, OP.mult, OP.min)
                nc.scalar.activation(out=e, in_=e, func=AF.Exp)
                # os_rho * ln1p(e) via nested Horner, one STT per step
                cs = [float(os_rho * c) for c in LN1P]
                nc.vector.tensor_scalar_mul(p, e, cs[0])
                for ck in cs[1:-1]:
                    nc.vector.scalar_tensor_tensor(p, p, ck, e,
                                                   OP.add, OP.mult)
                nc.vector.tensor_scalar_add(p, p, cs[-1])
                nc.vector.tensor_scalar(r, x, float(os_rho), 0.0,
                                        OP.mult, OP.max)
                nc.vector.tensor_add(s["fT"][:, 1, fsl], p, r)

            def dec_mm(b, k):
                s = st[b]
                kt = s[f"k2t_{k}"]
                nch = NCH_D[k]
                # alternate pools: psE is idle by now, so dec_mm gets an
                # effective 4-deep accumulator rotation
                if (2 * b + k) % 2 == 0:
                    msps = psD.tile([2, TGTU], f32, tag="D", name="ms_ps")
                else:
                    msps = psE.tile([2, TGTU], f32, tag="E", name="ms_ps")
                for c in range(nch):
                    nc.tensor.matmul(
                        msps,
                        s["fT"][:, :, J0S[k] + c],
                        kt[:, TGTU * c:TGTU * (c + 1)],
                        start=(c == 0), stop=(c == nch - 1),
                    )
                nc.vector.tensor_copy(
                    s["osl"][:, TGTU * k:TGTU * (k + 1)], msps)
                if k == 3:
                    nc.sync.dma_start(out=OUTh[b, :, 0:4 * TGTU],
                                      in_=s["osl"][:, 0:4 * TGTU])
                elif k == NBLK_D - 1:
                    nc.sync.dma_start(out=OUTh[b, :, 4 * TGTU:T_GRID],
                                      in_=s["osl"][:, 4 * TGTU:T_GRID])

            # ---------------- emission ----------------
            dec_units = [(b, k) for b in range(BLOC)
                         for k in range(NBLK_D)]
            du = [0]

            def emit_dec(nu=1):
                # deprioritized: dec K-tiles should fill conv-phase gaps, not
                # get hoisted ahead of the encoder where they stall on DMAs
                for _ in range(nu):
                    if du[0] < len(dec_units):
                        b, k = dec_units[du[0]]
                        with tc.high_priority(offset=-100000):
                            dec_block(b, k)
                        du[0] += 1

            for k in range(NBLK_E):
                enc_block(0, k)
                if k == 8:
                    epilogue(0, "A")
            epilogue(0, "B")
            for k in range(NBLK_E):
                enc_block(1, k)
                if k == 8:
                    epilogue(1, "A")
                    stack_shift(C1S, 32, 4, 0)
                    stack_shift(C1S, 32, 4, 1)
            epilogue(1, "B")
            stack_shift(C1S, 32, 4, 2)
            stack_shift(C1S, 32, 4, 3)

            nexts = {0: (F2, 32, 32), 2: (F4, 32, 32)}
            for l in range(4):
                for n in range(4):
                    conv_chunk(l, n)
                    if l < 3 and n >= 1:
                        if l == 1:
                            shift_f3(n - 1)
                        else:
                            stack_shift(*nexts[l][:2], nexts[l][2], n - 1)
                    emit_dec(1)
                    if l == 3 and n == 1:
                        fchain(0, 0)
                        fchain(1, 0)
                        for k in range(3):
                            dec_mm(0, k)
                            dec_mm(1, k)
                if l < 3:
                    if l == 1:
                        shift_f3(3)
                    else:
                        stack_shift(*nexts[l][:2], nexts[l][2], 3)
            emit_dec(len(dec_units))    # drain any remainder

            fchain(0, 1)
            fchain(1, 1)
            for k in range(3, NBLK_D):
                dec_mm(0, k)
                dec_mm(1, k)

    nc.compile()
    return nc


def _hi_lo(vals):
    """Split into f16-exact hi (multiples of 1/16) + small f16 lo."""
    f16, f64 = np.float16, np.float64
    hi = (np.round(np.asarray(vals, f64) * 16.0) / 16.0).astype(f16)
    lo = (np.asarray(vals, f64) - hi.astype(f64)).astype(f16)
    return hi, lo


def make_inmaps(inputs):
    f32 = np.float32
    f16 = np.float16
    f64 = np.float64
    xc = np.asarray(inputs["xc"])[..., 0].astype(f32)
    yc = np.asarray(inputs["yc"])[..., 0].astype(f32)
    xt = np.asarray(inputs["xt"])[..., 0].astype(f32)
    ls_psi = f64(np.float32(inputs["ls_psi"]))
    os_psi = f64(np.float32(inputs["os_psi"]))
    ls_rho = f64(np.float32(inputs["ls_rho"]))
    os_rho = f64(np.float32(inputs["os_rho"]))
    w = [np.asarray(inputs[f"w{i}"]).astype(f32) for i in (1, 2, 3, 4)]
    bs = [np.asarray(inputs[f"b{i}"]).astype(f32) for i in (1, 2, 3, 4)]

    lower = np.minimum(xc.min(), xt.min())
    upper = np.maximum(xc.max(), xt.max())
    t64 = np.linspace(f64(lower), f64(upper), T_GRID)
    delta = (t64[-1] - t64[0]) / (T_GRID - 1)

    a_psi = -0.5 / (ls_psi * ls_psi)
    a_rho = -0.5 / (ls_rho * ls_rho)
    m_psi = np.sqrt(ETH / -a_psi)
    m_rho = np.sqrt(ETH / -a_rho)

    perm_c = np.argsort(xc, axis=1, kind="stable")
    xcs = np.take_along_axis(xc, perm_c, 1).astype(f64)
    ycs = np.take_along_axis(yc, perm_c, 1).astype(f64)
    perm_t = np.argsort(xt, axis=1, kind="stable")
    xts = np.take_along_axis(xt, perm_t, 1).astype(f64)

    # encoder windows (16 blocks of 128 grid points)
    eidx = np.zeros((B, NBLK_E, 2), np.int64)
    for k in range(NBLK_E):
        lo = t64[WBLK_E * k] - m_psi
        hi = t64[WBLK_E * (k + 1) - 1] + m_psi
        for b in range(B):
            eidx[b, k, 0] = np.searchsorted(xcs[b], lo)
            eidx[b, k, 1] = np.searchsorted(xcs[b], hi)
    ecnt = eidx[:, :, 1] - eidx[:, :, 0]
    NCH_E = [max(1, int(np.ceil(ecnt[:, k].max() / 128)))
             for k in range(NBLK_E)]
    assert max(NCH_E) <= 4, NCH_E

    # decoder grid-chunk windows per xt quantile-block
    J0S, J1S = [], []
    for k in range(NBLK_D):
        xmin = min(xts[b, TGTU * k] for b in range(B))
        xmax = max(xts[b, TGTU * (k + 1) - 1] for b in range(B))
        g0 = max(0, int(np.searchsorted(t64, xmin - m_rho)) - 1)
        g1 = min(T_GRID - 1, int(np.searchsorted(t64, xmax + m_rho)))
        J0S.append(g0 // 128)
        J1S.append(g1 // 128 + 1)
    NCH_D = [J1S[k] - J0S[k] for k in range(NBLK_D)]
    assert max(NCH_D) <= 4, NCH_D
    SE = sum(NCH_E)
    MAXNE = max(NCH_E)
    RE = 2 + 5 * MAXNE
    BW = MAXNE * WBLK_E
    KGWC = BW + 2 * NBLK_D * 128
    XBC = NBLK_E * 128 + NBLK_D * 2 * TGTU

    tpr = (np.arange(WBLK_E) - (WBLK_E - 1) / 2.0) * delta
    te2_hi, te2_lo = _hi_lo(a_psi * tpr * tpr)
    th_hi, th_lo = _hi_lo(tpr)

    # KGW: [BDE | TDB0 | TDB1]
    KGW = np.zeros((17, KGWC), f16)
    for c in range(MAXNE):
        sl = slice(WBLK_E * c, WBLK_E * (c + 1))
        KGW[0, sl] = te2_hi
        KGW[1, sl] = te2_lo
        KGW[2 + 5 * c, sl] = 1
        KGW[3 + 5 * c, sl] = 1
        KGW[4 + 5 * c, sl] = th_hi
        KGW[5 + 5 * c, sl] = th_lo
        KGW[6 + 5 * c, sl] = th_hi
    for k in range(NBLK_D):
        gv = t64[128 * J0S[k]:128 * J1S[k]]
        cb = (gv[0] + gv[-1]) / 2.0
        tv = gv - cb
        for half in range(2):
            o = BW + 1024 * half
            ksl = slice(o + 128 * k, o + 128 * (k + 1))
            KGW[0:2, ksl] = 1
        for c in range(NCH_D[k]):
            half, cc = divmod(c, 2)
            o = BW + 1024 * half
            ksl = slice(o + 128 * k, o + 128 * (k + 1))
            tvc = tv[128 * c:128 * (c + 1)]
            gb_hi, gb_lo = _hi_lo(a_rho * tvc * tvc)
            v_hi, v_lo = _hi_lo(-2.0 * a_rho * tvc)
            KGW[2 + 5 * cc, ksl] = gb_hi
            KGW[3 + 5 * cc, ksl] = gb_lo
            KGW[4 + 5 * cc, ksl] = v_hi
            KGW[5 + 5 * cc, ksl] = v_hi
            KGW[6 + 5 * cc, ksl] = v_lo

    # conv1 t channel: affine in t -> 2 static rows + bias + edge fix
    t_hi, t_lo = _hi_lo(t64)
    TROW = np.stack([t_hi, t_lo], 0)
    A1 = w[0][:, 0, :].astype(f64).sum(1)
    C1 = bs[0].astype(f64) + delta * (w[0][:, 0, :].astype(f64)
                                      * (np.arange(5) - 2)).sum(1)
    L, U = t64[0], t64[-1]
    CR = np.zeros((32, 4), f64)
    w10 = w[0][:, 0, :].astype(f64)
    for half in range(2):
        r = slice(16 * half, 16 * half + 16)
        CR[r, 0] = -w10[:, 0] * (L - 2 * delta) - w10[:, 1] * (L - delta)
        CR[r, 1] = -w10[:, 0] * (L - delta)
        CR[r, 2] = -w10[:, 4] * (U + delta)
        CR[r, 3] = -w10[:, 3] * (U + delta) - w10[:, 4] * (U + 2 * delta)

    # block-diagonal batched conv weights, packed into WALL [128, 332]
    W1n = np.zeros((128, 32), f16)
    W1n[4, :] = np.tile(A1.astype(f16), 2)
    W1n[5, :] = np.tile(A1.astype(f16), 2)
    for o in range(4):
        for half in range(2):
            W1n[32 * o + 2 * half, 16 * half:16 * half + 16] = \
                w[0][:, 1, o].astype(f16)
            W1n[32 * o + 1 + 2 * half, 16 * half:16 * half + 16] = \
                w[0][:, 2, o].astype(f16)
    W1e = np.zeros((4, 32), f16)
    for half in range(2):
        W1e[2 * half, 16 * half:16 * half + 16] = w[0][:, 1, 4].astype(f16)
        W1e[1 + 2 * half, 16 * half:16 * half + 16] = w[0][:, 2, 4].astype(f16)
    W2n = np.zeros((128, 64), f16)
    for o in range(4):
        for half in range(2):
            W2n[32 * o + 16 * half:32 * o + 16 * half + 16,
                32 * half:32 * half + 32] = w[1][:, :, o].T.astype(f16)
    W2e = np.zeros((32, 64), f16)
    for half in range(2):
        W2e[16 * half:16 * half + 16, 32 * half:32 * half + 32] = \
            w[1][:, :, 4].T.astype(f16)
    W3n = np.zeros((128, 32), f16)
    for o in range(2):
        for half in range(2):
            W3n[64 * o + 32 * half:64 * o + 32 * half + 32,
                16 * half:16 * half + 16] = w[2][:, :, o].T.astype(f16)
    W3b = np.zeros((128, 32), f16)
    for o in (2, 3):
        for half in range(2):
            W3b[64 * (o - 2) + 32 * half:64 * (o - 2) + 32 * half + 32,
                16 * half:16 * half + 16] = w[2][:, :, o].T.astype(f16)
    W3e4 = np.zeros((64, 32), f16)
    for half in range(2):
        W3e4[32 * half:32 * half + 32, 16 * half:16 * half + 16] = \
            w[2][:, :, 4].T.astype(f16)
    W4n = np.zeros((128, 4), f16)
    for o in range(4):
        for half in range(2):
            W4n[32 * o + 16 * half:32 * o + 16 * half + 16,
                2 * half:2 * half + 2] = w[3][:, :, o].T.astype(f16)
    W4e = np.zeros((32, 4), f16)
    for half in range(2):
        W4e[16 * half:16 * half + 16, 2 * half:2 * half + 2] = \
            w[3][:, :, 4].T.astype(f16)
    WALL = np.zeros((128, 332), f16)
    WALL[0:128, 0:32] = W1n
    WALL[0:4, 32:64] = W1e
    WALL[0:128, 64:128] = W2n
    WALL[0:32, 128:192] = W2e
    WALL[0:128, 192:224] = W3n
    WALL[0:128, 224:256] = W3b
    WALL[0:64, 256:288] = W3e4
    WALL[0:128, 320:324] = W4n
    WALL[0:32, 324:328] = W4e
    WALL[0:4, 328:332] = np.eye(4, dtype=f16)

    BALL = np.zeros((64, 8), f32)
    BALL[0:64, 0] = np.concatenate([bs[1], bs[1]])
    BALL[0:32, 1] = np.concatenate([bs[2], bs[2]])
    BALL[0:32, 2] = np.concatenate([C1, C1]).astype(f32)
    BALL[0:32, 3:7] = CR.astype(f32)

    shared = {"KGW": KGW, "WALL": WALL, "BALL": BALL, "TROW": TROW}

    in_maps = []
    for core in range(NCORES):
        m = dict(shared)
        XB = np.zeros((BLOC, 17, XBC), f16)
        PHI = np.zeros((BLOC, 128, 2 * SE), f16)
        for bb in range(BLOC):
            b = core * BLOC + bb
            XB[bb, 0:2, 0:NBLK_E * 128] = 1
            base = 0
            for k in range(NBLK_E):
                ck = (t64[WBLK_E * k] + t64[WBLK_E * (k + 1) - 1]) / 2.0
                i0, i1 = eidx[b, k]
                nv = int(i1 - i0)
                ns = 128 * NCH_E[k]
                xv = np.zeros(ns, f64)
                xv[:nv] = xcs[b, i0:i1] - ck
                bias = np.full(ns, -60.0, f64)
                bias[:nv] = a_psi * xv[:nv] * xv[:nv]
                uv = np.zeros(ns, f64)
                uv[:nv] = -2.0 * a_psi * xv[:nv]
                ph = np.zeros((ns, 2), f64)
                ph[:nv, 0] = os_psi
                ph[:nv, 1] = os_psi * ycs[b, i0:i1]
                ksl = slice(128 * k, 128 * (k + 1))
                for c in range(NCH_E[k]):
                    sl = slice(128 * c, 128 * (c + 1))
                    b_hi, b_lo = _hi_lo(bias[sl])
                    u_hi, u_lo = _hi_lo(uv[sl])
                    XB[bb, 2 + 5 * c, ksl] = b_hi
                    XB[bb, 3 + 5 * c, ksl] = b_lo
                    XB[bb, 4 + 5 * c, ksl] = u_hi
                    XB[bb, 5 + 5 * c, ksl] = u_hi
                    XB[bb, 6 + 5 * c, ksl] = u_lo
                    PHI[bb, :, 2 * (base + c)] = ph[sl, 0].astype(f16)
                    PHI[bb, :, 2 * (base + c) + 1] = ph[sl, 1].astype(f16)
                base += NCH_E[k]
            for k in range(NBLK_D):
                gv = t64[128 * J0S[k]:128 * J1S[k]]
                cb = (gv[0] + gv[-1]) / 2.0
                i0, i1 = TGTU * k, TGTU * (k + 1)
                assert xts[b, i0] - m_rho >= gv[0] - delta or J0S[k] == 0
                assert xts[b, i1 - 1] + m_rho <= gv[-1] + delta \
                    or J1S[k] == 16
                xv = xts[b, i0:i1] - cb
                xb_hi, xb_lo = _hi_lo(a_rho * xv * xv)
                x_hi, x_lo = _hi_lo(xv)
                k0 = NBLK_E * 128 + 2 * TGTU * k
                for cc in range(2):
                    csl = slice(k0 + TGTU * cc, k0 + TGTU * (cc + 1))
                    XB[bb, 0, csl] = xb_hi
                    XB[bb, 1, csl] = xb_lo
                    XB[bb, 2 + 5 * cc, csl] = 1
                    XB[bb, 3 + 5 * cc, csl] = 1
                    XB[bb, 4 + 5 * cc, csl] = x_hi
                    XB[bb, 5 + 5 * cc, csl] = x_lo
                    XB[bb, 6 + 5 * cc, csl] = x_hi
        m["XB"] = XB
        m["PHI"] = PHI
        in_maps.append(m)

    cfg = {
        "NCH_E": NCH_E, "NCH_D": NCH_D, "J0S": J0S,
        "os_rho": float(os_rho), "b4_0": float(bs[3][0]),
        "b4_1": float(bs[3][1]),
    }
    aux = {"perm_t": perm_t}
    return in_maps, cfg, aux


def kernel(**inputs):
    from concourse.bass_utils import run_bass_kernel_spmd

    in_maps, cfg, aux = make_inmaps(inputs)
    key = (tuple(cfg["NCH_E"]), tuple(cfg["NCH_D"]), tuple(cfg["J0S"]),
           cfg["os_rho"], cfg["b4_0"], cfg["b4_1"])
    if key not in _PROG_CACHE:
        _PROG_CACHE[key] = build_program(cfg)
    nc = _PROG_CACHE[key]

    res = run_bass_kernel_spmd(nc, in_maps, core_ids=list(range(NCORES)))
    outs = [np.asarray(res.results[i]["out"]) for i in range(NCORES)]
    packed = np.concatenate(outs, 0)  # [B, 2, N] in sorted-xt order
    out = np.zeros((B, N, 2), np.float32)
    for b in range(B):
        out[b, aux["perm_t"][b], 0] = packed[b, 0]
        out[b, aux["perm_t"][b], 1] = packed[b, 1]
    return out
